# revision 19
# baseline (speedup 1.0000x reference)
"""Trainium2 Bass kernel for nn_Druggability_DistillModel (gnn_message_passing).

Strategy (8 NeuronCores, data-parallel over B x 4-way sequence shards):
  - core c handles batch b=c//4, tokens [s*512, (s+1)*512) with s=c%4.
  - The edge-bias MLP depends only on rel_pos (65 values) -> host collapses
    it to a table and builds LT[j, t] = log(sum_dup exp(edge)) over neighbors
    (−1e30 where none), so softmax_k(q.k/16 + edge) * v becomes
    exp(q.hK^T + LT) @ hV / rowsum — dense PE work, no gather.
  - LT is folded into the score PSUM via an identity matmul, so the sweep is
    matmul→matmul→matmul→Exp with no elementwise hop in between.
  - Denominators accumulate as rows of one [16,512] PSUM tile (one matmul per
    j-tile) and reduce with a single ones^T matmul at the end.
  - ACT table discipline: the scalar engine only ever loads the exp set (up
    front, via a dummy op that overlaps the first DMAs) and the gelu set (for
    the tail: gelu + tanh-as-sigmoid + square).  All rsqrt work (both
    layernorms) runs on the DVE as Heron iterations seeded from (1+v)/2.
  - PE warm-up: a burst of identity matmuls at t~0 lifts the HAM clock gate
    to full rate before the real prework arrives.
"""
import sys

sys.path.insert(0, "/opt/trn_rl_repo")

import math
import numpy as np
import ml_dtypes

B, L, D, H, DH, K, DE, CLIP = 2, 2048, 256, 8, 32, 36, 64, 32
NCORES, SPB, SH = 8, 4, 512  # cores, shards/batch, tokens/shard
NT = L // 128                # 16 token tiles per batch
ST = SH // 128               # 4 tiles per shard
BF16 = ml_dtypes.bfloat16

_CACHE: dict = {}


def _gelu_np(x):
    try:
        from scipy.special import erf
        e = erf(x / np.sqrt(2.0))
    except Exception:
        import math as _m
        e = np.vectorize(_m.erf)(x / np.sqrt(2.0))
    return x * 0.5 * (1.0 + e)


def _w_tiles(w, cin_chunks):
    """[din, dout] -> [128, cin_chunks, dout] with din = c*128+p."""
    din, dout = w.shape
    assert din == cin_chunks * 128
    return np.ascontiguousarray(
        w.reshape(cin_chunks, 128, dout).transpose(1, 0, 2)
    ).astype(BF16)


def _build(taps=()):
    import concourse.bass as bass
    import concourse.tile as tile
    from concourse import bacc, mybir
    from concourse.masks import make_identity

    f32, bf = mybir.dt.float32, mybir.dt.bfloat16
    AF = mybir.ActivationFunctionType
    ALU = mybir.AluOpType
    AX = mybir.AxisListType

    nc = bacc.Bacc("TRN2", target_bir_lowering=False, debug=False)

    x_d = nc.dram_tensor("x", [L, D], f32, kind="ExternalInput")
    lt_d = nc.dram_tensor("lt", [L, SH], bf, kind="ExternalInput")
    aff_d = nc.dram_tensor("aff", [128, 2, 4], f32, kind="ExternalInput")
    wq_d = nc.dram_tensor("wq", [128, 2, D], bf, kind="ExternalInput")
    wk_d = nc.dram_tensor("wk", [128, 2, D], bf, kind="ExternalInput")
    wv_d = nc.dram_tensor("wv", [128, 2, D], bf, kind="ExternalInput")
    wg1_d = nc.dram_tensor("wg1", [128, 4, D], bf, kind="ExternalInput")
    wg2_d = nc.dram_tensor("wg2", [128, 2, D], bf, kind="ExternalInput")
    wqkv_d = nc.dram_tensor("wqkv", [128, 2, 3 * D], bf, kind="ExternalInput")
    wgo_d = nc.dram_tensor("wgo", [128, 2, D], bf, kind="ExternalInput")
    wf1_d = nc.dram_tensor("wf1", [128, 2, D], bf, kind="ExternalInput")
    wf2_d = nc.dram_tensor("wf2", [128, 2, 2], bf, kind="ExternalInput")
    wff1_d = nc.dram_tensor("wff1", [128, 2, 4 * D], bf, kind="ExternalInput")
    wff2_d = nc.dram_tensor("wff2", [128, 8, D], bf, kind="ExternalInput")
    pm_d = nc.dram_tensor("pm", [2, 1], bf, kind="ExternalInput")
    out_d = nc.dram_tensor("out", [SH, D], f32, kind="ExternalOutput")
    tap_tiles = {}

    with tile.TileContext(nc) as tc:
        with (
            tc.tile_pool(name="const", bufs=1) as const,
            tc.tile_pool(name="persist", bufs=1) as pers,
            tc.tile_pool(name="stream", bufs=4) as stm,
            tc.tile_pool(name="stmf", bufs=4) as stmf,
            tc.tile_pool(name="stmq", bufs=6) as stmq,
            tc.tile_pool(name="psmm", bufs=3, space="PSUM") as psmm,
            tc.tile_pool(name="psacc", bufs=4, space="PSUM") as psacc,
            tc.tile_pool(name="pssml", bufs=1, space="PSUM") as pssml,
        ):
            ident = const.tile([128, 128], f32)
            make_identity(nc, ident[:])
            ident_bf = const.tile([128, 128], bf)
            make_identity(nc, ident_bf[:])
            ones_cb = const.tile([128, 1], bf)
            nc.vector.memset(ones_cb[:], 1.0)
            ones_rb = const.tile([1, 128], bf)
            nc.vector.memset(ones_rb[:], 1.0)
            pm = const.tile([2, 1], bf)
            nc.sync.dma_start(pm[:], pm_d[:])
            eps5 = const.tile([128, 1], f32)
            nc.vector.memset(eps5[:], 1e-5)
            ones_f1 = const.tile([1, 1], f32)
            nc.vector.memset(ones_f1[:], 1.0)
            aff = const.tile([128, 2, 4], f32)

            # preload the EXP activation table while DMAs stream in
            scr_e = const.tile([1, 1], f32)
            nc.scalar.activation(scr_e[:], eps5[0:1, 0:1], AF.Exp)


            x_all = pers.tile([128, NT, D], f32)
            x_r = x_d.rearrange("(n p) d -> p n d", p=128)
            nc.sync.dma_start(x_all[:, 0:1, :], x_r[:, 0:1, :])
            nc.sync.dma_start(x_all[:, 1:4, :], x_r[:, 1:4, :])
            nc.sync.dma_start(aff[:], aff_d[:])

            def wload(dram, shape):
                t = const.tile(list(shape), bf, tag=dram.name)
                nc.sync.dma_start(t[:], dram[:])
                return t

            wv = wload(wv_d, (128, 2, D))
            wqkv = wload(wqkv_d, (128, 2, 3 * D))
            for qg_ in range(1, 4):
                nc.sync.dma_start(x_all[:, qg_ * 4:(qg_ + 1) * 4, :],
                                  x_r[:, qg_ * 4:(qg_ + 1) * 4, :])
            wk = wload(wk_d, (128, 2, D))
            wq = wload(wq_d, (128, 2, D))
            lt_r = lt_d.rearrange("(n p) t -> p n t", p=128)
            lt_all = pers.tile([128, NT, SH], bf)
            for qg_ in range(4):
                nc.sync.dma_start(lt_all[:, qg_ * 4:(qg_ + 1) * 4, :],
                                  lt_r[:, qg_ * 4:(qg_ + 1) * 4, :])
            wf1 = wload(wf1_d, (128, 2, D))
            wf2 = wload(wf2_d, (128, 2, 2))
            wg1 = wload(wg1_d, (128, 4, D))
            wg2 = wload(wg2_d, (128, 2, D))
            wgo = wload(wgo_d, (128, 2, D))
            wff1 = wload(wff1_d, (128, 2, 4 * D))
            wff2 = wload(wff2_d, (128, 8, D))

            hT = pers.tile([128, 2, L], bf)    # h^T, full batch
            hKT = pers.tile([128, 2, L], bf)   # (h@Wk)^T, full batch
            hV = pers.tile([128, NT, D], bf)   # h@Wv@Wlo, token-major
            tap_tiles["hT"], tap_tiles["hKT"], tap_tiles["hV"] = hT, hKT, hV
            qT = pers.tile([128, 2, SH], bf)
            tap_tiles["qT"] = qT

            f1T = pers.tile([128, 2, SH], bf)
            qg_all = pers.tile([128, ST, D], f32)
            kv_ps = [psacc.tile([128, 257], f32, tag="acc", name=f"kv{g}")
                     for g in range(2)]
            agg_ps = [psacc.tile([128, 512], f32, tag="acc", name=f"agg{g}")
                      for g in range(2)]
            den_acc = pssml.tile([1, 512], f32, tag="accs", name="den")

            rstd_rest = pers.tile([128, 12], f32)
            nmr_rest = pers.tile([128, 12], f32)

            heron_n = [0]

            def heron_core(rstd_out, nmr_out, mean_ap, var_ap, iters, k):
                """rstd = 1/sqrt(var + 1e-5), nmr = -mean*rstd.  All-DVE
                Heron iterations (no ACT sqrt table)."""
                heron_n[0] += 1
                hid = heron_n[0]
                vh = stm.tile([128, k], f32, tag="her", name=f"vh_{hid}")
                nc.vector.tensor_scalar(vh[:], var_ap, 0.5, 5e-6,
                                        op0=ALU.mult, op1=ALU.add)
                s = stm.tile([128, k], f32, tag="her", name=f"s_{hid}")
                nc.vector.tensor_scalar_add(s[:], vh[:], 0.5)
                r = stm.tile([128, k], f32, tag="her", name=f"r_{hid}")
                q = stm.tile([128, k], f32, tag="her", name=f"q_{hid}")
                for _ in range(iters):
                    nc.vector.reciprocal(r[:], s[:])
                    nc.vector.tensor_mul(q[:], vh[:], r[:])
                    nc.vector.scalar_tensor_tensor(s[:], s[:], 0.5, q[:],
                                                   op0=ALU.mult, op1=ALU.add)
                nc.vector.reciprocal(rstd_out, s[:])
                nc.vector.scalar_tensor_tensor(nmr_out, mean_ap, -1.0,
                                               rstd_out, op0=ALU.mult,
                                               op1=ALU.mult)

            def heron_rstd(rstd_out, nmr_out, mean_c, msq_c, iters):
                k = mean_c.shape[-1]
                hid = heron_n[0] + 100
                m2 = stm.tile([128, k], f32, tag="her", name=f"m2_{hid}")
                nc.vector.tensor_mul(m2[:], mean_c[:], mean_c[:])
                df = stm.tile([128, k], f32, tag="her", name=f"df_{hid}")
                nc.vector.tensor_sub(df[:], msq_c[:], m2[:])
                heron_core(rstd_out, nmr_out, mean_c[:], df[:], iters, k)

            # producers for the software-pipelined accumulators
            kg_tiles = {}
            ut_tiles = {}

            def emit_kv(n):
                kg_l, vg_rhs = kg_tiles.pop(n)
                for g in range(2):
                    nc.tensor.matmul(kv_ps[g][:], kg_l[:, g * 128:(g + 1) * 128],
                                     vg_rhs[:], start=(n == 0), stop=(n == NT - 1))

            def emit_attn_acc(jc):
                ut = ut_tiles.pop(jc)
                for g in range(2):
                    nc.tensor.matmul(agg_ps[g][:], hV[:, jc, g * 128:(g + 1) * 128],
                                     ut[:], start=(jc == 0), stop=(jc == NT - 1))

            # ---------- fused pre-work + attention, per 4-tile group ----------
            for qgrp in range(4):
                tiles = range(qgrp * 4, qgrp * 4 + 4)
                if qgrp == 0:
                    mval = stm.tile([128, 4, 2], f32, tag="mval")
                    for i, n in enumerate(tiles):
                        stats = stm.tile([128, 6], f32, tag="stats")
                        nc.vector.bn_stats(out=stats[:], in_=x_all[:, n, :])
                        nc.vector.bn_aggr(out=mval[:, i, :], in_=stats[:])
                    rstd4 = stm.tile([128, 4], f32, tag="rstd4")
                    nmr4 = stm.tile([128, 4], f32, tag="nmr4")
                    heron_core(rstd4[:], nmr4[:], mval[:, :, 0], mval[:, :, 1],
                               iters=3, k=4)
                else:
                    rstd4 = rstd_rest[:, (qgrp - 1) * 4:qgrp * 4]
                    nmr4 = nmr_rest[:, (qgrp - 1) * 4:qgrp * 4]
                for i, n in enumerate(tiles):
                    js = slice(n * 128, (n + 1) * 128)
                    # hn = (x - m) * rstd  (one fused DVE op, bf16 out)
                    hn = stmq.tile([128, D], f32, tag="tmpq")
                    nc.vector.tensor_scalar(hn[:], x_all[:, n, :],
                                            rstd4[:, i:i + 1], nmr4[:, i:i + 1],
                                            op0=ALU.mult, op1=ALU.add)
                    for c in range(2):
                        pt = psmm.tile([128, 128], f32, tag="mm")
                        nc.tensor.transpose(pt[:], hn[:, c * 128:(c + 1) * 128],
                                            ident[:])
                        # h = hn * g1 + b1 on the transposed copy-out (DVE)
                        nc.vector.tensor_scalar(hT[:, c, js], pt[:],
                                                aff[:, c, 0:1], aff[:, c, 1:2],
                                                op0=ALU.mult, op1=ALU.add)
                    # hV tile
                    pv = psmm.tile([128, D], f32, tag="mm")
                    for c in range(2):
                        nc.tensor.matmul(pv[:], hT[:, c, js], wv[:, c, :],
                                         start=(c == 0), stop=(c == 1))
                    nc.scalar.copy(hV[:, n, :], pv[:])
                    # kg/vg projection + elu(k)+1 = min(exp(k),1) + max(k,0)
                    pq = psmm.tile([128, 512], f32, tag="mm")
                    for c in range(2):
                        nc.tensor.matmul(pq[:], hT[:, c, js], wqkv[:, c, D:3 * D],
                                         start=(c == 0), stop=(c == 1))
                    te = stmq.tile([128, D], bf, tag="tmpq")
                    nc.scalar.activation(te[:], pq[:, 0:D], AF.Exp)
                    ta_ = stmq.tile([128, D], bf, tag="tmpq")
                    nc.vector.tensor_scalar_min(ta_[:], te[:], 1.0)
                    tr = stmq.tile([128, D], bf, tag="tmpq")
                    nc.vector.tensor_scalar_max(tr[:], pq[:, 0:D], 0.0)
                    kg_l = stm.tile([128, D], bf, tag="kg_l")
                    nc.gpsimd.tensor_add(kg_l[:], ta_[:], tr[:])
                    vg_rhs = stm.tile([128, D + 1], bf, tag="vg_rhs")
                    nc.vector.tensor_copy(vg_rhs[:, 0:D], pq[:, D:2 * D])
                    nc.gpsimd.memset(vg_rhs[:, D:D + 1], 1.0)
                    kg_tiles[n] = (kg_l, vg_rhs)
                    if n >= 2:
                        emit_kv(n - 2)

                # hKT chunk for this group
                jsg = slice(qgrp * 512, (qgrp + 1) * 512)
                for g in range(2):
                    pk = psmm.tile([128, 512], f32, tag="mm")
                    for c in range(2):
                        nc.tensor.matmul(pk[:], wk[:, c, g * 128:(g + 1) * 128],
                                         hT[:, c, jsg], start=(c == 0), stop=(c == 1))
                    nc.scalar.copy(hKT[:, g, jsg], pk[:])
                # qT + linear-attn qg (needs hT tiles 0..3 only)
                if qgrp == 0:
                    for g in range(2):
                        pq2 = psmm.tile([128, 512], f32, tag="mm")
                        for c in range(2):
                            nc.tensor.matmul(pq2[:], wq[:, c, g * 128:(g + 1) * 128],
                                             hT[:, c, 0:SH], start=(c == 0), stop=(c == 1))
                        nc.vector.tensor_copy(qT[:, g, :], pq2[:])
                    for it in range(ST):
                        ts_ = slice(it * 128, (it + 1) * 128)
                        pq3 = psmm.tile([128, D], f32, tag="mm")
                        for c in range(2):
                            nc.tensor.matmul(pq3[:], hT[:, c, ts_], wqkv[:, c, 0:D],
                                             start=(c == 0), stop=(c == 1))
                        teb = stmq.tile([128, D], f32, tag="tmpq")
                        nc.scalar.activation(teb[:], pq3[:], AF.Exp)
                        tab_ = stmq.tile([128, D], f32, tag="tmpq")
                        nc.vector.tensor_scalar_min(tab_[:], teb[:], 1.0)
                        trb = stmq.tile([128, D], f32, tag="tmpq")
                        nc.vector.tensor_scalar_max(trb[:], pq3[:], 0.0)
                        nc.gpsimd.tensor_add(qg_all[:, it, :], tab_[:], trb[:])
                    # batched LN stats for tiles 4..15 (DVE bn + Heron)
                    mv_r = stm.tile([128, 12, 2], f32, tag="mv_r")
                    for i2, n2 in enumerate(range(4, NT)):
                        stats2 = stm.tile([128, 6], f32, tag="stats")
                        nc.vector.bn_stats(out=stats2[:], in_=x_all[:, n2, :])
                        nc.vector.bn_aggr(out=mv_r[:, i2, :], in_=stats2[:])
                    heron_core(rstd_rest[:], nmr_rest[:], mv_r[:, :, 0],
                               mv_r[:, :, 1], iters=3, k=12)

                # attention chunks for this group (acc pipelined one behind)
                for jc in tiles:
                    js = slice(jc * 128, (jc + 1) * 128)
                    pl = psmm.tile([128, 512], f32, tag="mm")
                    nc.tensor.matmul(pl[:], hKT[:, 0, js], qT[:, 0, :],
                                     start=True, stop=False)
                    nc.tensor.matmul(pl[:], hKT[:, 1, js], qT[:, 1, :],
                                     start=False, stop=False)
                    nc.tensor.matmul(pl[:], ident_bf[:], lt_all[:, jc, :],
                                     start=False, stop=True)
                    ut = stm.tile([128, 512], bf, tag="ut")
                    nc.scalar.activation(ut[:], pl[:], AF.Exp)
                    nc.tensor.matmul(den_acc[:], ones_cb[:], ut[:],
                                     start=(jc == 0), stop=(jc == NT - 1))
                    ut_tiles[jc] = ut
                    if jc >= 2:
                        emit_attn_acc(jc - 2)
            emit_kv(NT - 2)
            emit_kv(NT - 1)
            emit_attn_acc(NT - 2)
            emit_attn_acc(NT - 1)

            # ---------- denominator accumulated in PSUM during the sweep ----
            den_sb2 = stm.tile([1, 512], f32, tag="den_sb2")
            nc.vector.tensor_copy(den_sb2[:], den_acc[:])
            den_rb = pers.tile([1, 512], bf)
            tap_tiles["den_rb"] = den_rb
            with nc.allow_low_precision("bf16 recip feeds bf16 broadcast"):
                nc.vector.reciprocal(den_rb[:], den_sb2[:])

            # ---------- kv block-diagonal matrix + ksum row ----------
            kvb = pers.tile([128, 2, D], bf)
            tap_tiles["kvb"] = kvb
            nc.vector.memset(kvb[:], 0.0)
            for h in range(H):
                g, po = h // 4, (h * DH) % 128
                nc.scalar.copy(kvb[po:po + DH, g, h * DH:(h + 1) * DH],
                               kv_ps[g][po:po + DH, h * DH:(h + 1) * DH])
            ksum_col = pers.tile([128, 2], f32)
            for g in range(2):
                nc.vector.tensor_copy(ksum_col[:, g:g + 1], kv_ps[g][:, D:D + 1])
            ksum_row = pers.tile([1, D], bf)
            for g in range(2):
                pt = psmm.tile([128, 128], f32, tag="mm")
                nc.tensor.transpose(pt[0:1, 0:128], ksum_col[:, g:g + 1], ident[:])
                nc.vector.tensor_copy(ksum_row[0:1, g * 128:(g + 1) * 128], pt[0:1, 0:128])
            kb_ps = psmm.tile([128, D], f32, tag="mm")
            nc.tensor.matmul(kb_ps[:], ones_rb[:], ksum_row[:], start=True, stop=True)
            ksumb = pers.tile([128, D], bf)
            tap_tiles["ksumb"] = ksumb
            nc.vector.tensor_copy(ksumb[:], kb_ps[:])

            # ---------- tail (ACT: gelu set only — gelu/tanh/square/copy) -----
            aggloT = pers.tile([128, 2, SH], bf)
            tap_tiles["aggloT"] = aggloT
            g1T = pers.tile([128, 2, SH], bf)
            tgate = pers.tile([128, 2, SH], bf)
            tap_tiles["tgate"] = tgate
            h_localT = pers.tile([128, 2, SH], f32)
            tap_tiles["h_localT"] = h_localT
            qgzT = pers.tile([128, 2, SH], bf)
            tap_tiles["qgzT"] = qgzT
            yT = pers.tile([128, 2, SH], bf)
            tap_tiles["yT"] = yT
            h_globalT = pers.tile([128, 2, SH], f32)
            tap_tiles["h_globalT"] = h_globalT
            xoT = pers.tile([128, 2, SH], f32)
            tap_tiles["xoT"] = xoT
            xo_bf = pers.tile([128, 2, SH], bf)
            xnT = pers.tile([128, 2, SH], bf)
            tap_tiles["xnT"] = xnT
            ff1T = pers.tile([128, 8, SH], bf)
            outT = pers.tile([128, 2, SH], f32)
            tap_tiles["outT"] = outT

            # agglo = agg_un * recip(den): rb broadcast then per-group TT
            rbp = psmm.tile([128, 512], f32, tag="mm", name="rbp")
            nc.tensor.matmul(rbp[:], ones_rb[:], den_rb[:], start=True, stop=True)
            rbh = stmf.tile([128, 512], bf, tag="tmpf", name="rbh")
            nc.scalar.copy(rbh[:], rbp[:])
            for g in range(2):
                nc.vector.tensor_mul(aggloT[:, g, :], agg_ps[g][:], rbh[:])

            # f1 / fuse-gate chain (hT only; overlaps gate chain)
            for g in range(2):
                pf = psmm.tile([128, 512], f32, tag="mm")
                for c in range(2):
                    nc.tensor.matmul(pf[:], wf1[:, c, g * 128:(g + 1) * 128],
                                     hT[:, c, 0:SH], start=(c == 0), stop=(c == 1))
                nc.scalar.activation(f1T[:, g, :], pf[:], AF.Gelu)
            wf_ps = pssml.tile([2, 512], f32, tag="accs", name="wfps")
            wf_sb = stm.tile([2, 512], bf, tag="wf_sb")
            for c in range(2):
                nc.tensor.matmul(wf_ps[:], wf2[:, c, :], f1T[:, c, :],
                                 start=(c == 0), stop=(c == 1))
            nc.scalar.copy(wf_sb[:], wf_ps[:])
            d01_ps = psmm.tile([1, 512], f32, tag="mm", name="d01")
            nc.tensor.matmul(d01_ps[:], pm[:], wf_sb[:], start=True, stop=True)
            # sigmoid via tanh (gelu-set resident): s(x) = 0.5 + 0.5*tanh(x/2)
            th_wf = stm.tile([1, 512], bf, tag="th_wf")
            nc.scalar.activation(th_wf[:], d01_ps[:], AF.Tanh, scale=0.5)
            wf0 = pers.tile([1, 512], bf)
            tap_tiles["wf0"] = wf0
            wf1s = pers.tile([1, 512], bf)
            nc.vector.tensor_scalar(wf0[:], th_wf[:], 0.5, 0.5,
                                    op0=ALU.mult, op1=ALU.add)
            nc.vector.tensor_scalar(wf1s[:], th_wf[:], -0.5, 0.5,
                                    op0=ALU.mult, op1=ALU.add)

            # gate chain
            for g in range(2):
                pg = psmm.tile([128, 512], f32, tag="mm")
                for c in range(2):
                    nc.tensor.matmul(pg[:], wg1[:, c, g * 128:(g + 1) * 128],
                                     hT[:, c, 0:SH], start=(c == 0), stop=False)
                for c in range(2):
                    nc.tensor.matmul(pg[:], wg1[:, 2 + c, g * 128:(g + 1) * 128],
                                     aggloT[:, c, :], start=False, stop=(c == 1))
                nc.scalar.activation(g1T[:, g, :], pg[:], AF.Gelu)
            for g in range(2):
                pg2 = psmm.tile([128, 512], f32, tag="mm")
                for c in range(2):
                    nc.tensor.matmul(pg2[:], wg2[:, c, g * 128:(g + 1) * 128],
                                     g1T[:, c, :], start=(c == 0), stop=(c == 1))
                nc.scalar.activation(tgate[:, g, :], pg2[:], AF.Tanh, scale=0.5)
            # h_local = h + sigmoid(gate)*agglo = h + 0.5*(agglo + agglo*tanh)
            for g in range(2):
                u = stmf.tile([128, 512], bf, tag="tmpf")
                nc.gpsimd.tensor_mul(u[:], tgate[:, g, :], aggloT[:, g, :])
                v = stmf.tile([128, 512], bf, tag="tmpf")
                nc.gpsimd.tensor_add(v[:], aggloT[:, g, :], u[:])
                nc.vector.scalar_tensor_tensor(h_localT[:, g, :], v[:], 0.5,
                                               hT[:, g, 0:SH],
                                               op0=ALU.mult, op1=ALU.add)

            # linear attention z + qgz
            zden_a = stm.tile([128, ST, H], f32, tag="zden_a")
            for it in range(ST):
                prod = stmq.tile([128, D], f32, tag="tmpq")
                nc.vector.tensor_mul(prod[:], qg_all[:, it, :], ksumb[:])
                nc.vector.tensor_reduce(zden_a[:, it, :],
                                        prod[:].rearrange("p (h d) -> p h d", d=DH),
                                        axis=AX.X, op=ALU.add)
            nc.vector.tensor_scalar_add(zden_a[:], zden_a[:], 1e-6)
            zr_a = stm.tile([128, ST, H], f32, tag="zr_a")
            nc.vector.reciprocal(
                zr_a[:].rearrange("p a b -> p (a b)"),
                zden_a[:].rearrange("p a b -> p (a b)"))
            for it in range(ST):
                ts_ = slice(it * 128, (it + 1) * 128)
                qgz = stmq.tile([128, D], f32, tag="tmpq")
                nc.vector.tensor_tensor(
                    out=qgz[:].rearrange("p (h d) -> p h d", d=DH),
                    in0=qg_all[:, it, :].rearrange("p (h d) -> p h d", d=DH),
                    in1=zr_a[:, it, :].to_broadcast([128, H, DH]), op=ALU.mult)
                for c in range(2):
                    ptq = psmm.tile([128, 128], f32, tag="mm")
                    nc.tensor.transpose(ptq[:], qgz[:, c * 128:(c + 1) * 128],
                                        ident[:])
                    nc.vector.tensor_copy(qgzT[:, c, ts_], ptq[:])
            for g in range(2):
                py = psmm.tile([128, 512], f32, tag="mm")
                for c in range(2):
                    nc.tensor.matmul(py[:], kvb[:, c, g * 128:(g + 1) * 128],
                                     qgzT[:, c, :], start=(c == 0), stop=(c == 1))
                nc.vector.tensor_copy(yT[:, g, :], py[:])
            for g in range(2):
                pgo = psmm.tile([128, 512], f32, tag="mm")
                for c in range(2):
                    nc.tensor.matmul(pgo[:], wgo[:, c, g * 128:(g + 1) * 128],
                                     yT[:, c, :], start=(c == 0), stop=(c == 1))
                nc.vector.tensor_add(h_globalT[:, g, :], hT[:, g, 0:SH], pgo[:])

            # xo = wf0*h_local + wf1*h_global
            b0p = psmm.tile([128, 512], f32, tag="mm", name="b0p")
            nc.tensor.matmul(b0p[:], ones_rb[:], wf0[:], start=True, stop=True)
            b1p = psmm.tile([128, 512], f32, tag="mm", name="b1p")
            nc.tensor.matmul(b1p[:], ones_rb[:], wf1s[:], start=True, stop=True)
            for g in range(2):
                ta = stmf.tile([128, 512], f32, tag="tmpf")
                nc.vector.tensor_mul(ta[:], h_localT[:, g, :], b0p[:])
                tb = stmf.tile([128, 512], f32, tag="tmpf")
                nc.vector.tensor_mul(tb[:], h_globalT[:, g, :], b1p[:])
                nc.vector.tensor_add(xoT[:, g, :], ta[:], tb[:])
                nc.vector.tensor_copy(xo_bf[:, g, :], xoT[:, g, :])

            # ---------- LN2: stats via matmul, rsqrt via column Heron ---------
            sum_ps = pssml.tile([1, 512], f32, tag="accs", name="sumps")
            for c in range(2):
                nc.tensor.matmul(sum_ps[:], ones_cb[:], xo_bf[:, c, :],
                                 start=(c == 0), stop=(c == 1))
            ssq_ps = psmm.tile([1, 512], f32, tag="mm", name="ssqps")
            for c in range(2):
                xsq = stmf.tile([128, 512], bf, tag="xsq")
                nc.scalar.activation(xsq[:], xoT[:, c, :], AF.Square)
                nc.tensor.matmul(ssq_ps[:], ones_cb[:], xsq[:],
                                 start=(c == 0), stop=(c == 1))
            sum_row = stm.tile([1, 512], f32, tag="sum_row")
            nc.vector.tensor_copy(sum_row[:], sum_ps[:])
            ssq_row = stm.tile([1, 512], f32, tag="ssq_row")
            nc.vector.tensor_copy(ssq_row[:], ssq_ps[:])
            sq_col = stm.tile([128, 4, 2], f32, tag="sq_col")
            for kk in range(4):
                pts = psmm.tile([128, 128], f32, tag="mm")
                nc.tensor.transpose(pts[0:128, 0:1],
                                    sum_row[0:1, kk * 128:(kk + 1) * 128],
                                    ident[0:1, 0:1])
                nc.vector.tensor_copy(sq_col[:, kk, 0:1], pts[0:128, 0:1])
                ptq2 = psmm.tile([128, 128], f32, tag="mm")
                nc.tensor.transpose(ptq2[0:128, 0:1],
                                    ssq_row[0:1, kk * 128:(kk + 1) * 128],
                                    ident[0:1, 0:1])
                nc.vector.tensor_copy(sq_col[:, kk, 1:2], ptq2[0:128, 0:1])
            mean_c = stm.tile([128, 4], f32, tag="mean_c")
            nc.vector.tensor_scalar_mul(mean_c[:], sq_col[:, :, 0], 1.0 / D)
            msq_c = stm.tile([128, 4], f32, tag="msq_c")
            nc.vector.tensor_scalar_mul(msq_c[:], sq_col[:, :, 1], 1.0 / D)
            rstd_c = stm.tile([128, 4], f32, tag="rstd_c")
            nmr_c = stm.tile([128, 4], f32, tag="nmr_c")
            heron_rstd(rstd_c[:], nmr_c[:], mean_c, msq_c, iters=5)
            nmr_row = stm.tile([1, 512], bf, tag="nmr_row")
            rstd_row = stm.tile([1, 512], bf, tag="rstd_row")
            for kk in range(4):
                ptb0 = psmm.tile([128, 128], f32, tag="mm")
                nc.tensor.transpose(ptb0[0:1, 0:128], nmr_c[:, kk:kk + 1],
                                    ident[:])
                nc.vector.tensor_copy(nmr_row[:, kk * 128:(kk + 1) * 128],
                                      ptb0[0:1, 0:128])
                ptb1 = psmm.tile([128, 128], f32, tag="mm")
                nc.tensor.transpose(ptb1[0:1, 0:128], rstd_c[:, kk:kk + 1],
                                    ident[:])
                nc.vector.tensor_copy(rstd_row[:, kk * 128:(kk + 1) * 128],
                                      ptb1[0:1, 0:128])
            nmr_b = psmm.tile([128, 512], f32, tag="mm", name="nmrb")
            nc.tensor.matmul(nmr_b[:], ones_rb[:], nmr_row[:],
                             start=True, stop=True)
            rb2_b = psmm.tile([128, 512], f32, tag="mm", name="rb2b")
            nc.tensor.matmul(rb2_b[:], ones_rb[:], rstd_row[:],
                             start=True, stop=True)
            for g in range(2):
                t1 = stmf.tile([128, 512], f32, tag="tmpf")
                nc.vector.tensor_mul(t1[:], xoT[:, g, :], rb2_b[:])
                # g2 is folded into Wff1 on the host; write bf16 directly
                nc.vector.tensor_add(xnT[:, g, :], t1[:], nmr_b[:])

            # FFN + residual + output transposes/stores
            for g8 in range(8):
                pff = psmm.tile([128, 512], f32, tag="mm")
                for c in range(2):
                    nc.tensor.matmul(pff[:], wff1[:, c, g8 * 128:(g8 + 1) * 128],
                                     xnT[:, c, :], start=(c == 0), stop=(c == 1))
                nc.scalar.activation(ff1T[:, g8, :], pff[:], AF.Gelu)
            for g in range(2):
                pf2 = psmm.tile([128, 512], f32, tag="mm")
                for c8 in range(8):
                    nc.tensor.matmul(pf2[:], wff2[:, c8, g * 128:(g + 1) * 128],
                                     ff1T[:, c8, :], start=(c8 == 0), stop=(c8 == 7))
                nc.vector.tensor_add(outT[:, g, :], xoT[:, g, :], pf2[:])
            for it in range(ST):
                ts_ = slice(it * 128, (it + 1) * 128)
                ot = stmq.tile([128, D], f32, tag="tmpq")
                for c in range(2):
                    pto = psmm.tile([128, 128], f32, tag="mm")
                    nc.tensor.transpose(pto[:], outT[:, c, ts_], ident[:])
                    nc.vector.tensor_copy(ot[:, c * 128:(c + 1) * 128], pto[:])
                nc.sync.dma_start(out_d[ts_, :], ot[:])

            for name in taps:
                t = tap_tiles[name]
                td = nc.dram_tensor(f"tap_{name}", list(t.shape),
                                    t.dtype, kind="ExternalOutput")
                nc.sync.dma_start(td[:], t[:])

    nc.compile()
    return nc


def _host_prep(inputs):
    """Host-side preprocessing shared by all cores + per-core arrays."""
    x = np.asarray(inputs["x"], np.float32)
    mask = np.asarray(inputs["mask"])
    nbr_idx = np.asarray(inputs["nbr_idx"]).astype(np.int64)
    nbr_mask = np.asarray(inputs["nbr_mask"])
    rel_pos = np.asarray(inputs["rel_pos"]).astype(np.int64)

    if not (np.all(mask == 1)):
        raise NotImplementedError("kernel assumes mask == ones (spec fill)")

    # edge-bias table over the 65 possible rel values
    Erel = np.asarray(inputs["Erel"], np.float32)
    We1 = np.asarray(inputs["We1"], np.float32)
    be1 = np.asarray(inputs["be1"], np.float32)
    We2 = np.asarray(inputs["We2"], np.float32)
    be2 = np.asarray(inputs["be2"], np.float32)
    tab = (_gelu_np(Erel @ We1 + be1) @ We2 + be2)[:, 0]  # [65]

    rel = np.clip(rel_pos, -CLIP, CLIP) + CLIP
    ev = np.exp(tab[rel]) * (nbr_mask != 0)  # [B, L, K]

    # dense E^T per batch: ET[b][j, t] = sum_k ev[b,t,k] * [idx==j]
    ET = np.zeros((B, L, L), np.float32)
    for b in range(B):
        t_idx = np.repeat(np.arange(L), K)
        np.add.at(ET[b], (nbr_idx[b].ravel(), t_idx), ev[b].ravel())
    # log-domain (folded into the score PSUM in-kernel): -1e30 where empty
    LT = np.where(ET > 0, np.log(np.maximum(ET, 1e-30)), -1e30).astype(np.float32)

    aff = np.zeros((128, 2, 4), np.float32)
    for name, i in (("g1", 0), ("b1", 1), ("g2", 2), ("b2", 3)):
        v = np.asarray(inputs[name], np.float32)
        aff[:, :, i] = v.reshape(2, 128).T

    shared = {
        "aff": aff,
        "wq": _w_tiles(np.asarray(inputs["Wq"], np.float32) / 16.0, 2),
        "wk": _w_tiles(np.asarray(inputs["Wk"], np.float32), 2),
        "wv": _w_tiles(np.asarray(inputs["Wv"], np.float32)
                       @ np.asarray(inputs["Wlo"], np.float32), 2),
        "wg1": _w_tiles(np.asarray(inputs["Wg1"], np.float32), 4),
        "wg2": _w_tiles(np.asarray(inputs["Wg2"], np.float32), 2),
        "wqkv": _w_tiles(np.asarray(inputs["Wqkv"], np.float32), 2),
        "wgo": _w_tiles(np.asarray(inputs["Wgo"], np.float32), 2),
        "wf1": _w_tiles(np.asarray(inputs["Wf1"], np.float32), 2),
        "wf2": _w_tiles(np.asarray(inputs["Wf2"], np.float32), 2),
        "wff1": _w_tiles(np.asarray(inputs["g2"], np.float32)[:, None]
                         * np.asarray(inputs["Wff1"], np.float32), 2),
        "wff2": _w_tiles(np.asarray(inputs["Wff2"], np.float32), 8),
        "pm": np.array([[1.0], [-1.0]], BF16),
    }
    for k in ("blo", "bg1", "bg2", "bf1", "bf2", "bff1", "bff2", "b2"):
        if not np.allclose(np.asarray(inputs[k]), 0.0):
            raise NotImplementedError(f"kernel assumes bias {k} == 0 (spec fill)")

    per_core = []
    for c in range(NCORES):
        b, s = c // SPB, c % SPB
        s0 = s * SH
        xp = np.roll(x[b], -s0, axis=0)
        ltp = np.roll(LT[b][:, s0:s0 + SH], -s0, axis=0).astype(BF16)
        per_core.append({"x": np.ascontiguousarray(xp),
                         "lt": np.ascontiguousarray(ltp)})
    return shared, per_core


def kernel(**inputs) -> np.ndarray:
    import concourse.bass_utils as bu

    if "nc" not in _CACHE:
        _CACHE["nc"] = _build()
    nc = _CACHE["nc"]

    shared, per_core = _host_prep(inputs)
    in_maps = [{**shared, **pc} for pc in per_core]
    res = bu.run_bass_kernel_spmd(nc, in_maps, core_ids=list(range(NCORES)))
    out = np.zeros((B, L, D), np.float32)
    for c in range(NCORES):
        b, s = c // SPB, c % SPB
        out[b, s * SH:(s + 1) * SH] = res.results[c]["out"]
    return out


# revision 21
# speedup vs baseline: 1.1320x; 1.1320x over previous
"""Trainium2 Bass kernel for nn_Druggability_DistillModel (gnn_message_passing).

Strategy (8 NeuronCores, data-parallel over B x 4-way sequence shards):
  - core c handles batch b=c//4, tokens [s*512, (s+1)*512) with s=c%4.
  - The edge-bias MLP depends only on rel_pos (65 values) -> host collapses
    it to a table and builds LT[j, t] = log(sum_dup exp(edge)) over neighbors
    (−1e30 where none), so softmax_k(q.k/16 + edge) * v becomes
    exp(q.hK^T + LT) @ hV / rowsum — dense PE work, no gather.
  - LT is folded into the score PSUM via an identity matmul, so the sweep is
    matmul→matmul→matmul→Exp with no elementwise hop in between.
  - Denominators accumulate as rows of one [16,512] PSUM tile (one matmul per
    j-tile) and reduce with a single ones^T matmul at the end.
  - ACT table discipline: the scalar engine only ever loads the exp set (up
    front, via a dummy op that overlaps the first DMAs) and the gelu set (for
    the tail: gelu + tanh-as-sigmoid + square).  All rsqrt work (both
    layernorms) runs on the DVE as Heron iterations seeded from (1+v)/2.
  - PE warm-up: a burst of identity matmuls at t~0 lifts the HAM clock gate
    to full rate before the real prework arrives.
"""
import sys

sys.path.insert(0, "/opt/trn_rl_repo")

import math
import numpy as np
import ml_dtypes

B, L, D, H, DH, K, DE, CLIP = 2, 2048, 256, 8, 32, 36, 64, 32
NCORES, SPB, SH = 8, 4, 512  # cores, shards/batch, tokens/shard
NT = L // 128                # 16 token tiles per batch
ST = SH // 128               # 4 tiles per shard
BF16 = ml_dtypes.bfloat16

_CACHE: dict = {}


def _gelu_np(x):
    try:
        from scipy.special import erf
        e = erf(x / np.sqrt(2.0))
    except Exception:
        import math as _m
        e = np.vectorize(_m.erf)(x / np.sqrt(2.0))
    return x * 0.5 * (1.0 + e)


def _w_tiles(w, cin_chunks):
    """[din, dout] -> [128, cin_chunks, dout] with din = c*128+p."""
    din, dout = w.shape
    assert din == cin_chunks * 128
    return np.ascontiguousarray(
        w.reshape(cin_chunks, 128, dout).transpose(1, 0, 2)
    ).astype(BF16)


def _build(taps=()):
    import concourse.bass as bass
    import concourse.tile as tile
    from concourse import bacc, mybir
    from concourse.masks import make_identity

    f32, bf = mybir.dt.float32, mybir.dt.bfloat16
    AF = mybir.ActivationFunctionType
    ALU = mybir.AluOpType
    AX = mybir.AxisListType

    nc = bacc.Bacc("TRN2", target_bir_lowering=False, debug=False)

    x_d = nc.dram_tensor("x", [L, D], f32, kind="ExternalInput")
    lt_d = nc.dram_tensor("lt", [L, SH], bf, kind="ExternalInput")
    aff_d = nc.dram_tensor("aff", [128, 2, 4], f32, kind="ExternalInput")
    wq_d = nc.dram_tensor("wq", [128, 2, D], bf, kind="ExternalInput")
    wk_d = nc.dram_tensor("wk", [128, 2, D], bf, kind="ExternalInput")
    wv_d = nc.dram_tensor("wv", [128, 2, D], bf, kind="ExternalInput")
    wg1_d = nc.dram_tensor("wg1", [128, 4, D], bf, kind="ExternalInput")
    wg2_d = nc.dram_tensor("wg2", [128, 2, D], bf, kind="ExternalInput")
    wqkv_d = nc.dram_tensor("wqkv", [128, 2, 3 * D], bf, kind="ExternalInput")
    wgo_d = nc.dram_tensor("wgo", [128, 2, D], bf, kind="ExternalInput")
    wf1_d = nc.dram_tensor("wf1", [128, 2, D], bf, kind="ExternalInput")
    wf2_d = nc.dram_tensor("wf2", [128, 2, 2], bf, kind="ExternalInput")
    wff1_d = nc.dram_tensor("wff1", [128, 2, 4 * D], bf, kind="ExternalInput")
    wff2_d = nc.dram_tensor("wff2", [128, 8, D], bf, kind="ExternalInput")
    pm_d = nc.dram_tensor("pm", [2, 1], bf, kind="ExternalInput")
    out_d = nc.dram_tensor("out", [SH, D], f32, kind="ExternalOutput")
    tap_tiles = {}

    with tile.TileContext(nc) as tc:
        with (
            tc.tile_pool(name="const", bufs=1) as const,
            tc.tile_pool(name="persist", bufs=1) as pers,
            tc.tile_pool(name="stream", bufs=4) as stm,
            tc.tile_pool(name="stmf", bufs=4) as stmf,
            tc.tile_pool(name="stmq", bufs=8) as stmq,
            tc.tile_pool(name="hnp", bufs=3) as hnp,
            tc.tile_pool(name="psmm", bufs=3, space="PSUM") as psmm,
            tc.tile_pool(name="psacc", bufs=4, space="PSUM") as psacc,
            tc.tile_pool(name="pssml", bufs=1, space="PSUM") as pssml,
        ):
            ident = const.tile([128, 128], f32)
            make_identity(nc, ident[:])
            ident_bf = const.tile([128, 128], bf)
            make_identity(nc, ident_bf[:])
            ones_cb = const.tile([128, 1], bf)
            nc.vector.memset(ones_cb[:], 1.0)
            ones_rb = const.tile([1, 128], bf)
            nc.vector.memset(ones_rb[:], 1.0)
            pm = const.tile([2, 1], bf)
            nc.sync.dma_start(pm[:], pm_d[:])
            eps5 = const.tile([128, 1], f32)
            nc.vector.memset(eps5[:], 1e-5)
            ones_f1 = const.tile([1, 1], f32)
            nc.vector.memset(ones_f1[:], 1.0)
            aff = const.tile([128, 2, 4], f32)

            # preload the EXP activation table while DMAs stream in
            scr_e = const.tile([1, 1], f32)
            nc.scalar.activation(scr_e[:], eps5[0:1, 0:1], AF.Exp)

            # PE warm-up: lift the HAM clock gate before real matmuls arrive;
            # results are read once by a dummy copy so the buffer has a reader
            warm_ps = psmm.tile([128, 128], f32, tag="mm", name="warm")
            for wi in range(20):
                nc.tensor.matmul(warm_ps[:], ident_bf[:], ident_bf[:],
                                 start=True, stop=True)
            warm_sb = const.tile([1, 1], f32)
            nc.vector.tensor_copy(warm_sb[:], warm_ps[0:1, 0:1])


            x_all = pers.tile([128, NT, D], f32)
            x_r = x_d.rearrange("(n p) d -> p n d", p=128)
            nc.sync.dma_start(x_all[:, 0:1, :], x_r[:, 0:1, :])
            nc.sync.dma_start(x_all[:, 1:4, :], x_r[:, 1:4, :])
            nc.sync.dma_start(aff[:], aff_d[:])

            def wload(dram, shape):
                t = const.tile(list(shape), bf, tag=dram.name)
                nc.sync.dma_start(t[:], dram[:])
                return t

            wv = wload(wv_d, (128, 2, D))
            wqkv = wload(wqkv_d, (128, 2, 3 * D))
            for qg_ in range(1, 4):
                nc.sync.dma_start(x_all[:, qg_ * 4:(qg_ + 1) * 4, :],
                                  x_r[:, qg_ * 4:(qg_ + 1) * 4, :])
            wk = wload(wk_d, (128, 2, D))
            wq = wload(wq_d, (128, 2, D))
            lt_r = lt_d.rearrange("(n p) t -> p n t", p=128)
            lt_all = pers.tile([128, NT, SH], bf)
            for qg_ in range(4):
                nc.sync.dma_start(lt_all[:, qg_ * 4:(qg_ + 1) * 4, :],
                                  lt_r[:, qg_ * 4:(qg_ + 1) * 4, :])
            wf1 = wload(wf1_d, (128, 2, D))
            wf2 = wload(wf2_d, (128, 2, 2))
            wg1 = wload(wg1_d, (128, 4, D))
            wg2 = wload(wg2_d, (128, 2, D))
            wgo = wload(wgo_d, (128, 2, D))
            wff1 = wload(wff1_d, (128, 2, 4 * D))
            wff2 = wload(wff2_d, (128, 8, D))

            hT = pers.tile([128, 2, L], bf)    # h^T, full batch
            hKT = pers.tile([128, 2, L], bf)   # (h@Wk)^T, full batch
            hV = pers.tile([128, NT, D], bf)   # h@Wv@Wlo, token-major
            tap_tiles["hT"], tap_tiles["hKT"], tap_tiles["hV"] = hT, hKT, hV
            qT = pers.tile([128, 2, SH], bf)
            tap_tiles["qT"] = qT

            f1T = pers.tile([128, 2, SH], bf)
            qg_all = pers.tile([128, ST, D], f32)
            kv_ps = [psacc.tile([128, 257], f32, tag="acc", name=f"kv{g}")
                     for g in range(2)]
            agg_ps = [psacc.tile([128, 512], f32, tag="acc", name=f"agg{g}")
                      for g in range(2)]
            den_acc = pssml.tile([1, 512], f32, tag="accs", name="den")

            rstd_rest = pers.tile([128, 12], f32)
            nmr_rest = pers.tile([128, 12], f32)

            heron_n = [0]

            def heron_core(rstd_out, nmr_out, mean_ap, var_ap, iters, k):
                """rstd = 1/sqrt(var + 1e-5), nmr = -mean*rstd.  All-DVE
                Heron iterations (no ACT sqrt table)."""
                heron_n[0] += 1
                hid = heron_n[0]
                vh = stm.tile([128, k], f32, tag="her", name=f"vh_{hid}")
                nc.vector.tensor_scalar(vh[:], var_ap, 0.5, 5e-6,
                                        op0=ALU.mult, op1=ALU.add)
                s = stm.tile([128, k], f32, tag="her", name=f"s_{hid}")
                nc.vector.tensor_scalar_add(s[:], vh[:], 0.5)
                r = stm.tile([128, k], f32, tag="her", name=f"r_{hid}")
                q = stm.tile([128, k], f32, tag="her", name=f"q_{hid}")
                for _ in range(iters):
                    nc.vector.reciprocal(r[:], s[:])
                    nc.vector.tensor_mul(q[:], vh[:], r[:])
                    nc.vector.scalar_tensor_tensor(s[:], s[:], 0.5, q[:],
                                                   op0=ALU.mult, op1=ALU.add)
                nc.vector.reciprocal(rstd_out, s[:])
                nc.vector.scalar_tensor_tensor(nmr_out, mean_ap, -1.0,
                                               rstd_out, op0=ALU.mult,
                                               op1=ALU.mult)

            def heron_rstd(rstd_out, nmr_out, mean_c, msq_c, iters):
                k = mean_c.shape[-1]
                hid = heron_n[0] + 100
                m2 = stm.tile([128, k], f32, tag="her", name=f"m2_{hid}")
                nc.vector.tensor_mul(m2[:], mean_c[:], mean_c[:])
                df = stm.tile([128, k], f32, tag="her", name=f"df_{hid}")
                nc.vector.tensor_sub(df[:], msq_c[:], m2[:])
                heron_core(rstd_out, nmr_out, mean_c[:], df[:], iters, k)

            # producers for the software-pipelined accumulators
            kg_tiles = {}
            ut_tiles = {}

            def emit_kv(n):
                kg_l, vg_rhs = kg_tiles.pop(n)
                for g in range(2):
                    nc.tensor.matmul(kv_ps[g][:], kg_l[:, g * 128:(g + 1) * 128],
                                     vg_rhs[:], start=(n == 0), stop=(n == NT - 1))

            def emit_attn_acc(jc):
                ut = ut_tiles.pop(jc)
                for g in range(2):
                    nc.tensor.matmul(agg_ps[g][:], hV[:, jc, g * 128:(g + 1) * 128],
                                     ut[:], start=(jc == 0), stop=(jc == NT - 1))

            # ---------- fused pre-work + attention, per 4-tile group ----------
            for qgrp in range(4):
                tiles = range(qgrp * 4, qgrp * 4 + 4)
                if qgrp == 0:
                    mval = stm.tile([128, 4, 2], f32, tag="mval")
                    rstd4 = stm.tile([128, 4], f32, tag="rstd4")
                    nmr4 = stm.tile([128, 4], f32, tag="nmr4")
                    stats = stm.tile([128, 6], f32, tag="stats")
                    nc.vector.bn_stats(out=stats[:], in_=x_all[:, 0, :])
                    nc.vector.bn_aggr(out=mval[:, 0, :], in_=stats[:])
                    heron_core(rstd4[:, 0:1], nmr4[:, 0:1], mval[:, 0, 0:1],
                               mval[:, 0, 1:2], iters=3, k=1)
                    for i, n in enumerate(tiles):
                        if i == 0:
                            continue
                        stats = stm.tile([128, 6], f32, tag="stats")
                        nc.vector.bn_stats(out=stats[:], in_=x_all[:, n, :])
                        nc.vector.bn_aggr(out=mval[:, i, :], in_=stats[:])
                    heron_core(rstd4[:, 1:4], nmr4[:, 1:4], mval[:, 1:4, 0],
                               mval[:, 1:4, 1], iters=3, k=3)
                else:
                    rstd4 = rstd_rest[:, (qgrp - 1) * 4:qgrp * 4]
                    nmr4 = nmr_rest[:, (qgrp - 1) * 4:qgrp * 4]
                for i, n in enumerate(tiles):
                    js = slice(n * 128, (n + 1) * 128)
                    # hn = (x - m) * rstd  (one fused DVE op, bf16 out)
                    hn = hnp.tile([128, D], f32, tag="hn")
                    nc.vector.tensor_scalar(hn[:], x_all[:, n, :],
                                            rstd4[:, i:i + 1], nmr4[:, i:i + 1],
                                            op0=ALU.mult, op1=ALU.add)
                    for c in range(2):
                        pt = psmm.tile([128, 128], f32, tag="mm")
                        nc.tensor.transpose(pt[:], hn[:, c * 128:(c + 1) * 128],
                                            ident[:])
                        # h = hn * g1 + b1 on the transposed copy-out (DVE)
                        nc.vector.tensor_scalar(hT[:, c, js], pt[:],
                                                aff[:, c, 0:1], aff[:, c, 1:2],
                                                op0=ALU.mult, op1=ALU.add)
                    # hV tile
                    pv = psmm.tile([128, D], f32, tag="mm")
                    for c in range(2):
                        nc.tensor.matmul(pv[:], hT[:, c, js], wv[:, c, :],
                                         start=(c == 0), stop=(c == 1))
                    nc.scalar.copy(hV[:, n, :], pv[:])
                    # kg/vg projection + elu(k)+1 = min(exp(k),1) + max(k,0)
                    pq = psmm.tile([128, 512], f32, tag="mm")
                    for c in range(2):
                        nc.tensor.matmul(pq[:], hT[:, c, js], wqkv[:, c, D:3 * D],
                                         start=(c == 0), stop=(c == 1))
                    te = stmq.tile([128, D], bf, tag="tmpq")
                    nc.scalar.activation(te[:], pq[:, 0:D], AF.Exp)
                    ta_ = stmq.tile([128, D], bf, tag="tmpq")
                    nc.vector.tensor_scalar_min(ta_[:], te[:], 1.0)
                    tr = stmq.tile([128, D], bf, tag="tmpq")
                    nc.vector.tensor_scalar_max(tr[:], pq[:, 0:D], 0.0)
                    kg_l = stm.tile([128, D], bf, tag="kg_l")
                    nc.vector.tensor_add(kg_l[:], ta_[:], tr[:])
                    vg_rhs = stm.tile([128, D + 1], bf, tag="vg_rhs")
                    nc.vector.tensor_copy(vg_rhs[:, 0:D], pq[:, D:2 * D])
                    nc.gpsimd.memset(vg_rhs[:, D:D + 1], 1.0)
                    kg_tiles[n] = (kg_l, vg_rhs)
                    if n >= 2:
                        emit_kv(n - 2)

                # hKT chunk for this group
                jsg = slice(qgrp * 512, (qgrp + 1) * 512)
                for g in range(2):
                    pk = psmm.tile([128, 512], f32, tag="mm")
                    for c in range(2):
                        nc.tensor.matmul(pk[:], wk[:, c, g * 128:(g + 1) * 128],
                                         hT[:, c, jsg], start=(c == 0), stop=(c == 1))
                    nc.scalar.copy(hKT[:, g, jsg], pk[:])
                # qT + linear-attn qg (needs hT tiles 0..3 only)
                if qgrp == 0:
                    for g in range(2):
                        pq2 = psmm.tile([128, 512], f32, tag="mm")
                        for c in range(2):
                            nc.tensor.matmul(pq2[:], wq[:, c, g * 128:(g + 1) * 128],
                                             hT[:, c, 0:SH], start=(c == 0), stop=(c == 1))
                        nc.vector.tensor_copy(qT[:, g, :], pq2[:])
                    for it in range(ST):
                        ts_ = slice(it * 128, (it + 1) * 128)
                        pq3 = psmm.tile([128, D], f32, tag="mm")
                        for c in range(2):
                            nc.tensor.matmul(pq3[:], hT[:, c, ts_], wqkv[:, c, 0:D],
                                             start=(c == 0), stop=(c == 1))
                        teb = stmq.tile([128, D], f32, tag="tmpq")
                        nc.scalar.activation(teb[:], pq3[:], AF.Exp)
                        tab_ = stmq.tile([128, D], f32, tag="tmpq")
                        nc.vector.tensor_scalar_min(tab_[:], teb[:], 1.0)
                        trb = stmq.tile([128, D], f32, tag="tmpq")
                        nc.vector.tensor_scalar_max(trb[:], pq3[:], 0.0)
                        nc.vector.tensor_add(qg_all[:, it, :], tab_[:], trb[:])
                    # batched LN stats for tiles 4..15 (DVE bn + Heron)
                    mv_r = stm.tile([128, 12, 2], f32, tag="mv_r")
                    for i2, n2 in enumerate(range(4, NT)):
                        stats2 = stm.tile([128, 6], f32, tag="stats")
                        nc.vector.bn_stats(out=stats2[:], in_=x_all[:, n2, :])
                        nc.vector.bn_aggr(out=mv_r[:, i2, :], in_=stats2[:])
                    heron_core(rstd_rest[:], nmr_rest[:], mv_r[:, :, 0],
                               mv_r[:, :, 1], iters=3, k=12)

                # attention chunks for this group (acc pipelined one behind)
                for jc in tiles:
                    js = slice(jc * 128, (jc + 1) * 128)
                    pl = psmm.tile([128, 512], f32, tag="mm")
                    nc.tensor.matmul(pl[:], hKT[:, 0, js], qT[:, 0, :],
                                     start=True, stop=False)
                    nc.tensor.matmul(pl[:], hKT[:, 1, js], qT[:, 1, :],
                                     start=False, stop=False)
                    nc.tensor.matmul(pl[:], ident_bf[:], lt_all[:, jc, :],
                                     start=False, stop=True)
                    ut = stm.tile([128, 512], bf, tag="ut")
                    nc.scalar.activation(ut[:], pl[:], AF.Exp)
                    nc.tensor.matmul(den_acc[:], ones_cb[:], ut[:],
                                     start=(jc == 0), stop=(jc == NT - 1))
                    ut_tiles[jc] = ut
                    if jc >= 2:
                        emit_attn_acc(jc - 2)
            emit_kv(NT - 2)
            emit_kv(NT - 1)
            emit_attn_acc(NT - 2)
            emit_attn_acc(NT - 1)

            # ---------- denominator accumulated in PSUM during the sweep ----
            den_sb2 = stm.tile([1, 512], f32, tag="den_sb2")
            nc.vector.tensor_copy(den_sb2[:], den_acc[:])
            den_rb = pers.tile([1, 512], bf)
            tap_tiles["den_rb"] = den_rb
            with nc.allow_low_precision("bf16 recip feeds bf16 broadcast"):
                nc.vector.reciprocal(den_rb[:], den_sb2[:])

            # ---------- kv block-diagonal matrix + ksum row ----------
            kvb = pers.tile([128, 2, D], bf)
            tap_tiles["kvb"] = kvb
            nc.vector.memset(kvb[:], 0.0)
            for h in range(H):
                g, po = h // 4, (h * DH) % 128
                nc.scalar.copy(kvb[po:po + DH, g, h * DH:(h + 1) * DH],
                               kv_ps[g][po:po + DH, h * DH:(h + 1) * DH])
            ksum_col = pers.tile([128, 2], f32)
            for g in range(2):
                nc.vector.tensor_copy(ksum_col[:, g:g + 1], kv_ps[g][:, D:D + 1])
            ksum_row = pers.tile([1, D], bf)
            for g in range(2):
                pt = psmm.tile([128, 128], f32, tag="mm")
                nc.tensor.transpose(pt[0:1, 0:128], ksum_col[:, g:g + 1], ident[:])
                nc.vector.tensor_copy(ksum_row[0:1, g * 128:(g + 1) * 128], pt[0:1, 0:128])
            kb_ps = psmm.tile([128, D], f32, tag="mm")
            nc.tensor.matmul(kb_ps[:], ones_rb[:], ksum_row[:], start=True, stop=True)
            ksumb = pers.tile([128, D], bf)
            tap_tiles["ksumb"] = ksumb
            nc.vector.tensor_copy(ksumb[:], kb_ps[:])

            # ---------- tail (ACT: gelu set only — gelu/tanh/square/copy) -----
            aggloT = pers.tile([128, 2, SH], bf)
            tap_tiles["aggloT"] = aggloT
            g1T = pers.tile([128, 2, SH], bf)
            tgate = pers.tile([128, 2, SH], bf)
            tap_tiles["tgate"] = tgate
            h_localT = pers.tile([128, 2, SH], f32)
            tap_tiles["h_localT"] = h_localT
            qgzT = pers.tile([128, 2, SH], bf)
            tap_tiles["qgzT"] = qgzT
            yT = pers.tile([128, 2, SH], bf)
            tap_tiles["yT"] = yT
            h_globalT = pers.tile([128, 2, SH], f32)
            tap_tiles["h_globalT"] = h_globalT
            xoT = pers.tile([128, 2, SH], f32)
            tap_tiles["xoT"] = xoT
            xo_bf = pers.tile([128, 2, SH], bf)
            xnT = pers.tile([128, 2, SH], bf)
            tap_tiles["xnT"] = xnT
            ff1T = pers.tile([128, 8, SH], bf)
            outT = pers.tile([128, 2, SH], f32)
            tap_tiles["outT"] = outT

            # agglo = agg_un * recip(den): rb broadcast then per-group TT
            rbp = psmm.tile([128, 512], f32, tag="mm", name="rbp")
            nc.tensor.matmul(rbp[:], ones_rb[:], den_rb[:], start=True, stop=True)
            rbh = stmf.tile([128, 512], bf, tag="tmpf", name="rbh")
            nc.scalar.copy(rbh[:], rbp[:])
            for g in range(2):
                nc.vector.tensor_mul(aggloT[:, g, :], agg_ps[g][:], rbh[:])

            # f1 / fuse-gate chain (hT only; overlaps gate chain)
            for g in range(2):
                pf = psmm.tile([128, 512], f32, tag="mm")
                for c in range(2):
                    nc.tensor.matmul(pf[:], wf1[:, c, g * 128:(g + 1) * 128],
                                     hT[:, c, 0:SH], start=(c == 0), stop=(c == 1))
                nc.scalar.activation(f1T[:, g, :], pf[:], AF.Gelu)
            wf_ps = pssml.tile([2, 512], f32, tag="accs", name="wfps")
            wf_sb = stm.tile([2, 512], bf, tag="wf_sb")
            for c in range(2):
                nc.tensor.matmul(wf_ps[:], wf2[:, c, :], f1T[:, c, :],
                                 start=(c == 0), stop=(c == 1))
            nc.scalar.copy(wf_sb[:], wf_ps[:])
            d01_ps = psmm.tile([1, 512], f32, tag="mm", name="d01")
            nc.tensor.matmul(d01_ps[:], pm[:], wf_sb[:], start=True, stop=True)
            # sigmoid via tanh (gelu-set resident): s(x) = 0.5 + 0.5*tanh(x/2)
            th_wf = stm.tile([1, 512], bf, tag="th_wf")
            nc.scalar.activation(th_wf[:], d01_ps[:], AF.Tanh, scale=0.5)
            wf0 = pers.tile([1, 512], bf)
            tap_tiles["wf0"] = wf0
            wf1s = pers.tile([1, 512], bf)
            nc.vector.tensor_scalar(wf0[:], th_wf[:], 0.5, 0.5,
                                    op0=ALU.mult, op1=ALU.add)
            nc.vector.tensor_scalar(wf1s[:], th_wf[:], -0.5, 0.5,
                                    op0=ALU.mult, op1=ALU.add)

            # gate chain
            for g in range(2):
                pg = psmm.tile([128, 512], f32, tag="mm")
                for c in range(2):
                    nc.tensor.matmul(pg[:], wg1[:, c, g * 128:(g + 1) * 128],
                                     hT[:, c, 0:SH], start=(c == 0), stop=False)
                for c in range(2):
                    nc.tensor.matmul(pg[:], wg1[:, 2 + c, g * 128:(g + 1) * 128],
                                     aggloT[:, c, :], start=False, stop=(c == 1))
                nc.scalar.activation(g1T[:, g, :], pg[:], AF.Gelu)
            for g in range(2):
                pg2 = psmm.tile([128, 512], f32, tag="mm")
                for c in range(2):
                    nc.tensor.matmul(pg2[:], wg2[:, c, g * 128:(g + 1) * 128],
                                     g1T[:, c, :], start=(c == 0), stop=(c == 1))
                nc.scalar.activation(tgate[:, g, :], pg2[:], AF.Tanh, scale=0.5)
            # h_local = h + sigmoid(gate)*agglo = h + 0.5*(agglo + agglo*tanh)
            for g in range(2):
                u = stmf.tile([128, 512], bf, tag="tmpf")
                nc.gpsimd.tensor_mul(u[:], tgate[:, g, :], aggloT[:, g, :])
                v = stmf.tile([128, 512], bf, tag="tmpf")
                nc.gpsimd.tensor_add(v[:], aggloT[:, g, :], u[:])
                nc.vector.scalar_tensor_tensor(h_localT[:, g, :], v[:], 0.5,
                                               hT[:, g, 0:SH],
                                               op0=ALU.mult, op1=ALU.add)

            # linear attention z + qgz
            zden_a = stm.tile([128, ST, H], f32, tag="zden_a")
            for it in range(ST):
                prod = stmq.tile([128, D], f32, tag="tmpq")
                nc.vector.tensor_mul(prod[:], qg_all[:, it, :], ksumb[:])
                nc.vector.tensor_reduce(zden_a[:, it, :],
                                        prod[:].rearrange("p (h d) -> p h d", d=DH),
                                        axis=AX.X, op=ALU.add)
            nc.vector.tensor_scalar_add(zden_a[:], zden_a[:], 1e-6)
            zr_a = stm.tile([128, ST, H], f32, tag="zr_a")
            nc.vector.reciprocal(
                zr_a[:].rearrange("p a b -> p (a b)"),
                zden_a[:].rearrange("p a b -> p (a b)"))
            for it in range(ST):
                ts_ = slice(it * 128, (it + 1) * 128)
                qgz = stmq.tile([128, D], f32, tag="tmpq")
                nc.vector.tensor_tensor(
                    out=qgz[:].rearrange("p (h d) -> p h d", d=DH),
                    in0=qg_all[:, it, :].rearrange("p (h d) -> p h d", d=DH),
                    in1=zr_a[:, it, :].to_broadcast([128, H, DH]), op=ALU.mult)
                for c in range(2):
                    ptq = psmm.tile([128, 128], f32, tag="mm")
                    nc.tensor.transpose(ptq[:], qgz[:, c * 128:(c + 1) * 128],
                                        ident[:])
                    nc.vector.tensor_copy(qgzT[:, c, ts_], ptq[:])
            for g in range(2):
                py = psmm.tile([128, 512], f32, tag="mm")
                for c in range(2):
                    nc.tensor.matmul(py[:], kvb[:, c, g * 128:(g + 1) * 128],
                                     qgzT[:, c, :], start=(c == 0), stop=(c == 1))
                nc.vector.tensor_copy(yT[:, g, :], py[:])
            for g in range(2):
                pgo = psmm.tile([128, 512], f32, tag="mm")
                for c in range(2):
                    nc.tensor.matmul(pgo[:], wgo[:, c, g * 128:(g + 1) * 128],
                                     yT[:, c, :], start=(c == 0), stop=(c == 1))
                nc.vector.tensor_add(h_globalT[:, g, :], hT[:, g, 0:SH], pgo[:])

            # xo = wf0*h_local + wf1*h_global
            b0p = psmm.tile([128, 512], f32, tag="mm", name="b0p")
            nc.tensor.matmul(b0p[:], ones_rb[:], wf0[:], start=True, stop=True)
            b1p = psmm.tile([128, 512], f32, tag="mm", name="b1p")
            nc.tensor.matmul(b1p[:], ones_rb[:], wf1s[:], start=True, stop=True)
            for g in range(2):
                ta = stmf.tile([128, 512], f32, tag="tmpf")
                nc.vector.tensor_mul(ta[:], h_localT[:, g, :], b0p[:])
                tb = stmf.tile([128, 512], f32, tag="tmpf")
                nc.vector.tensor_mul(tb[:], h_globalT[:, g, :], b1p[:])
                nc.vector.tensor_add(xoT[:, g, :], ta[:], tb[:])
                nc.vector.tensor_copy(xo_bf[:, g, :], xoT[:, g, :])

            # ---------- LN2: stats via matmul, rsqrt via column Heron ---------
            sum_ps = pssml.tile([1, 512], f32, tag="accs", name="sumps")
            for c in range(2):
                nc.tensor.matmul(sum_ps[:], ones_cb[:], xo_bf[:, c, :],
                                 start=(c == 0), stop=(c == 1))
            ssq_ps = psmm.tile([1, 512], f32, tag="mm", name="ssqps")
            for c in range(2):
                xsq = stmf.tile([128, 512], bf, tag="xsq")
                nc.scalar.activation(xsq[:], xoT[:, c, :], AF.Square)
                nc.tensor.matmul(ssq_ps[:], ones_cb[:], xsq[:],
                                 start=(c == 0), stop=(c == 1))
            sum_row = stm.tile([1, 512], f32, tag="sum_row")
            nc.vector.tensor_copy(sum_row[:], sum_ps[:])
            ssq_row = stm.tile([1, 512], f32, tag="ssq_row")
            nc.vector.tensor_copy(ssq_row[:], ssq_ps[:])
            sq_col = stm.tile([128, 4, 2], f32, tag="sq_col")
            for kk in range(4):
                pts = psmm.tile([128, 128], f32, tag="mm")
                nc.tensor.transpose(pts[0:128, 0:1],
                                    sum_row[0:1, kk * 128:(kk + 1) * 128],
                                    ident[0:1, 0:1])
                nc.vector.tensor_copy(sq_col[:, kk, 0:1], pts[0:128, 0:1])
                ptq2 = psmm.tile([128, 128], f32, tag="mm")
                nc.tensor.transpose(ptq2[0:128, 0:1],
                                    ssq_row[0:1, kk * 128:(kk + 1) * 128],
                                    ident[0:1, 0:1])
                nc.vector.tensor_copy(sq_col[:, kk, 1:2], ptq2[0:128, 0:1])
            mean_c = stm.tile([128, 4], f32, tag="mean_c")
            nc.vector.tensor_scalar_mul(mean_c[:], sq_col[:, :, 0], 1.0 / D)
            msq_c = stm.tile([128, 4], f32, tag="msq_c")
            nc.vector.tensor_scalar_mul(msq_c[:], sq_col[:, :, 1], 1.0 / D)
            rstd_c = stm.tile([128, 4], f32, tag="rstd_c")
            nmr_c = stm.tile([128, 4], f32, tag="nmr_c")
            heron_rstd(rstd_c[:], nmr_c[:], mean_c, msq_c, iters=5)
            nmr_row = stm.tile([1, 512], bf, tag="nmr_row")
            rstd_row = stm.tile([1, 512], bf, tag="rstd_row")
            for kk in range(4):
                ptb0 = psmm.tile([128, 128], f32, tag="mm")
                nc.tensor.transpose(ptb0[0:1, 0:128], nmr_c[:, kk:kk + 1],
                                    ident[:])
                nc.vector.tensor_copy(nmr_row[:, kk * 128:(kk + 1) * 128],
                                      ptb0[0:1, 0:128])
                ptb1 = psmm.tile([128, 128], f32, tag="mm")
                nc.tensor.transpose(ptb1[0:1, 0:128], rstd_c[:, kk:kk + 1],
                                    ident[:])
                nc.vector.tensor_copy(rstd_row[:, kk * 128:(kk + 1) * 128],
                                      ptb1[0:1, 0:128])
            nmr_b = psmm.tile([128, 512], f32, tag="mm", name="nmrb")
            nc.tensor.matmul(nmr_b[:], ones_rb[:], nmr_row[:],
                             start=True, stop=True)
            rb2_b = psmm.tile([128, 512], f32, tag="mm", name="rb2b")
            nc.tensor.matmul(rb2_b[:], ones_rb[:], rstd_row[:],
                             start=True, stop=True)
            for g in range(2):
                t1 = stmf.tile([128, 512], f32, tag="tmpf")
                nc.vector.tensor_mul(t1[:], xoT[:, g, :], rb2_b[:])
                # g2 is folded into Wff1 on the host; write bf16 directly
                nc.vector.tensor_add(xnT[:, g, :], t1[:], nmr_b[:])

            # FFN + residual + output transposes/stores
            pf2s = [psacc.tile([128, 512], f32, tag="acc", name=f"pf2_{g}")
                    for g in range(2)]
            for g8 in range(8):
                pff = psmm.tile([128, 512], f32, tag="mm")
                for c in range(2):
                    nc.tensor.matmul(pff[:], wff1[:, c, g8 * 128:(g8 + 1) * 128],
                                     xnT[:, c, :], start=(c == 0), stop=(c == 1))
                nc.scalar.activation(ff1T[:, g8, :], pff[:], AF.Gelu)
                if g8 >= 1:
                    for g in range(2):
                        nc.tensor.matmul(pf2s[g][:],
                                         wff2[:, g8 - 1, g * 128:(g + 1) * 128],
                                         ff1T[:, g8 - 1, :],
                                         start=(g8 == 1), stop=False)
            for g in range(2):
                nc.tensor.matmul(pf2s[g][:], wff2[:, 7, g * 128:(g + 1) * 128],
                                 ff1T[:, 7, :], start=False, stop=True)
                nc.vector.tensor_add(outT[:, g, :], xoT[:, g, :], pf2s[g][:])
            for it in range(ST):
                ts_ = slice(it * 128, (it + 1) * 128)
                ot = stmq.tile([128, D], f32, tag="tmpq")
                for c in range(2):
                    pto = psmm.tile([128, 128], f32, tag="mm")
                    nc.tensor.transpose(pto[:], outT[:, c, ts_], ident[:])
                    nc.vector.tensor_copy(ot[:, c * 128:(c + 1) * 128], pto[:])
                nc.sync.dma_start(out_d[ts_, :], ot[:])

            for name in taps:
                t = tap_tiles[name]
                td = nc.dram_tensor(f"tap_{name}", list(t.shape),
                                    t.dtype, kind="ExternalOutput")
                nc.sync.dma_start(td[:], t[:])

    nc.compile()
    return nc


def _host_prep(inputs):
    """Host-side preprocessing shared by all cores + per-core arrays."""
    x = np.asarray(inputs["x"], np.float32)
    mask = np.asarray(inputs["mask"])
    nbr_idx = np.asarray(inputs["nbr_idx"]).astype(np.int64)
    nbr_mask = np.asarray(inputs["nbr_mask"])
    rel_pos = np.asarray(inputs["rel_pos"]).astype(np.int64)

    if not (np.all(mask == 1)):
        raise NotImplementedError("kernel assumes mask == ones (spec fill)")

    # edge-bias table over the 65 possible rel values
    Erel = np.asarray(inputs["Erel"], np.float32)
    We1 = np.asarray(inputs["We1"], np.float32)
    be1 = np.asarray(inputs["be1"], np.float32)
    We2 = np.asarray(inputs["We2"], np.float32)
    be2 = np.asarray(inputs["be2"], np.float32)
    tab = (_gelu_np(Erel @ We1 + be1) @ We2 + be2)[:, 0]  # [65]

    rel = np.clip(rel_pos, -CLIP, CLIP) + CLIP
    ev = np.exp(tab[rel]) * (nbr_mask != 0)  # [B, L, K]

    # dense E^T per batch: ET[b][j, t] = sum_k ev[b,t,k] * [idx==j]
    ET = np.zeros((B, L, L), np.float32)
    for b in range(B):
        t_idx = np.repeat(np.arange(L), K)
        np.add.at(ET[b], (nbr_idx[b].ravel(), t_idx), ev[b].ravel())
    # log-domain (folded into the score PSUM in-kernel): -1e30 where empty
    LT = np.where(ET > 0, np.log(np.maximum(ET, 1e-30)), -1e30).astype(np.float32)

    aff = np.zeros((128, 2, 4), np.float32)
    for name, i in (("g1", 0), ("b1", 1), ("g2", 2), ("b2", 3)):
        v = np.asarray(inputs[name], np.float32)
        aff[:, :, i] = v.reshape(2, 128).T

    shared = {
        "aff": aff,
        "wq": _w_tiles(np.asarray(inputs["Wq"], np.float32) / 16.0, 2),
        "wk": _w_tiles(np.asarray(inputs["Wk"], np.float32), 2),
        "wv": _w_tiles(np.asarray(inputs["Wv"], np.float32)
                       @ np.asarray(inputs["Wlo"], np.float32), 2),
        "wg1": _w_tiles(np.asarray(inputs["Wg1"], np.float32), 4),
        "wg2": _w_tiles(np.asarray(inputs["Wg2"], np.float32), 2),
        "wqkv": _w_tiles(np.asarray(inputs["Wqkv"], np.float32), 2),
        "wgo": _w_tiles(np.asarray(inputs["Wgo"], np.float32), 2),
        "wf1": _w_tiles(np.asarray(inputs["Wf1"], np.float32), 2),
        "wf2": _w_tiles(np.asarray(inputs["Wf2"], np.float32), 2),
        "wff1": _w_tiles(np.asarray(inputs["g2"], np.float32)[:, None]
                         * np.asarray(inputs["Wff1"], np.float32), 2),
        "wff2": _w_tiles(np.asarray(inputs["Wff2"], np.float32), 8),
        "pm": np.array([[1.0], [-1.0]], BF16),
    }
    for k in ("blo", "bg1", "bg2", "bf1", "bf2", "bff1", "bff2", "b2"):
        if not np.allclose(np.asarray(inputs[k]), 0.0):
            raise NotImplementedError(f"kernel assumes bias {k} == 0 (spec fill)")

    per_core = []
    for c in range(NCORES):
        b, s = c // SPB, c % SPB
        s0 = s * SH
        xp = np.roll(x[b], -s0, axis=0)
        ltp = np.roll(LT[b][:, s0:s0 + SH], -s0, axis=0).astype(BF16)
        per_core.append({"x": np.ascontiguousarray(xp),
                         "lt": np.ascontiguousarray(ltp)})
    return shared, per_core


def kernel(**inputs) -> np.ndarray:
    import concourse.bass_utils as bu

    if "nc" not in _CACHE:
        _CACHE["nc"] = _build()
    nc = _CACHE["nc"]

    shared, per_core = _host_prep(inputs)
    in_maps = [{**shared, **pc} for pc in per_core]
    res = bu.run_bass_kernel_spmd(nc, in_maps, core_ids=list(range(NCORES)))
    out = np.zeros((B, L, D), np.float32)
    for c in range(NCORES):
        b, s = c // SPB, c % SPB
        out[b, s * SH:(s + 1) * SH] = res.results[c]["out"]
    return out


# revision 22
# speedup vs baseline: 1.1393x; 1.0064x over previous
"""Trainium2 Bass kernel for nn_Druggability_DistillModel (gnn_message_passing).

Strategy (8 NeuronCores, data-parallel over B x 4-way sequence shards):
  - core c handles batch b=c//4, tokens [s*512, (s+1)*512) with s=c%4.
  - The edge-bias MLP depends only on rel_pos (65 values) -> host collapses
    it to a table and builds LT[j, t] = log(sum_dup exp(edge)) over neighbors
    (−1e30 where none), so softmax_k(q.k/16 + edge) * v becomes
    exp(q.hK^T + LT) @ hV / rowsum — dense PE work, no gather.
  - LT is folded into the score PSUM via an identity matmul, so the sweep is
    matmul→matmul→matmul→Exp with no elementwise hop in between.
  - Denominators accumulate as rows of one [16,512] PSUM tile (one matmul per
    j-tile) and reduce with a single ones^T matmul at the end.
  - ACT table discipline: the scalar engine only ever loads the exp set (up
    front, via a dummy op that overlaps the first DMAs) and the gelu set (for
    the tail: gelu + tanh-as-sigmoid + square).  All rsqrt work (both
    layernorms) runs on the DVE as Heron iterations seeded from (1+v)/2.
  - PE warm-up: a burst of identity matmuls at t~0 lifts the HAM clock gate
    to full rate before the real prework arrives.
"""
import sys

sys.path.insert(0, "/opt/trn_rl_repo")

import math
import numpy as np
import ml_dtypes

B, L, D, H, DH, K, DE, CLIP = 2, 2048, 256, 8, 32, 36, 64, 32
NCORES, SPB, SH = 8, 4, 512  # cores, shards/batch, tokens/shard
NT = L // 128                # 16 token tiles per batch
ST = SH // 128               # 4 tiles per shard
BF16 = ml_dtypes.bfloat16

_CACHE: dict = {}


def _gelu_np(x):
    try:
        from scipy.special import erf
        e = erf(x / np.sqrt(2.0))
    except Exception:
        import math as _m
        e = np.vectorize(_m.erf)(x / np.sqrt(2.0))
    return x * 0.5 * (1.0 + e)


def _w_tiles(w, cin_chunks):
    """[din, dout] -> [128, cin_chunks, dout] with din = c*128+p."""
    din, dout = w.shape
    assert din == cin_chunks * 128
    return np.ascontiguousarray(
        w.reshape(cin_chunks, 128, dout).transpose(1, 0, 2)
    ).astype(BF16)


def _build(taps=()):
    import concourse.bass as bass
    import concourse.tile as tile
    from concourse import bacc, mybir
    from concourse.masks import make_identity

    f32, bf = mybir.dt.float32, mybir.dt.bfloat16
    AF = mybir.ActivationFunctionType
    ALU = mybir.AluOpType
    AX = mybir.AxisListType

    nc = bacc.Bacc("TRN2", target_bir_lowering=False, debug=False)

    x_d = nc.dram_tensor("x", [L, D], f32, kind="ExternalInput")
    lt_d = nc.dram_tensor("lt", [L, SH], bf, kind="ExternalInput")
    aff_d = nc.dram_tensor("aff", [128, 2, 4], f32, kind="ExternalInput")
    wq_d = nc.dram_tensor("wq", [128, 2, D], bf, kind="ExternalInput")
    wk_d = nc.dram_tensor("wk", [128, 2, D], bf, kind="ExternalInput")
    wv_d = nc.dram_tensor("wv", [128, 2, D], bf, kind="ExternalInput")
    wg1_d = nc.dram_tensor("wg1", [128, 4, D], bf, kind="ExternalInput")
    wg2_d = nc.dram_tensor("wg2", [128, 2, D], bf, kind="ExternalInput")
    wqkv_d = nc.dram_tensor("wqkv", [128, 2, 3 * D], bf, kind="ExternalInput")
    wgo_d = nc.dram_tensor("wgo", [128, 2, D], bf, kind="ExternalInput")
    wf1_d = nc.dram_tensor("wf1", [128, 2, D], bf, kind="ExternalInput")
    wf2_d = nc.dram_tensor("wf2", [128, 2, 2], bf, kind="ExternalInput")
    wff1_d = nc.dram_tensor("wff1", [128, 2, 4 * D], bf, kind="ExternalInput")
    wff2_d = nc.dram_tensor("wff2", [128, 8, D], bf, kind="ExternalInput")
    pm_d = nc.dram_tensor("pm", [2, 1], bf, kind="ExternalInput")
    out_d = nc.dram_tensor("out", [SH, D], f32, kind="ExternalOutput")
    tap_tiles = {}

    with tile.TileContext(nc) as tc:
        with (
            tc.tile_pool(name="const", bufs=1) as const,
            tc.tile_pool(name="persist", bufs=1) as pers,
            tc.tile_pool(name="stream", bufs=4) as stm,
            tc.tile_pool(name="stmf", bufs=4) as stmf,
            tc.tile_pool(name="stmq", bufs=8) as stmq,
            tc.tile_pool(name="hnp", bufs=3) as hnp,
            tc.tile_pool(name="psmm", bufs=3, space="PSUM") as psmm,
            tc.tile_pool(name="psacc", bufs=4, space="PSUM") as psacc,
            tc.tile_pool(name="pssml", bufs=1, space="PSUM") as pssml,
        ):
            ident = const.tile([128, 128], f32)
            make_identity(nc, ident[:])
            ident_bf = const.tile([128, 128], bf)
            make_identity(nc, ident_bf[:])
            ones_cb = const.tile([128, 1], bf)
            nc.vector.memset(ones_cb[:], 1.0)
            ones_rb = const.tile([1, 128], bf)
            nc.vector.memset(ones_rb[:], 1.0)
            pm = const.tile([2, 1], bf)
            nc.sync.dma_start(pm[:], pm_d[:])
            eps5 = const.tile([128, 1], f32)
            nc.vector.memset(eps5[:], 1e-5)
            ones_f1 = const.tile([1, 1], f32)
            nc.vector.memset(ones_f1[:], 1.0)
            aff = const.tile([128, 2, 4], f32)

            # preload the EXP activation table while DMAs stream in
            scr_e = const.tile([1, 1], f32)
            nc.scalar.activation(scr_e[:], eps5[0:1, 0:1], AF.Exp)

            # PE warm-up: lift the HAM clock gate before real matmuls arrive;
            # results are read once by a dummy copy so the buffer has a reader
            warm_ps = psmm.tile([128, 128], f32, tag="mm", name="warm")
            for wi in range(20):
                nc.tensor.matmul(warm_ps[:], ident_bf[:], ident_bf[:],
                                 start=True, stop=True)
            warm_sb = const.tile([1, 1], f32)
            nc.vector.tensor_copy(warm_sb[:], warm_ps[0:1, 0:1])


            x_all = pers.tile([128, NT, D], f32)
            x_r = x_d.rearrange("(n p) d -> p n d", p=128)
            nc.sync.dma_start(x_all[:, 0:1, :], x_r[:, 0:1, :])
            nc.sync.dma_start(x_all[:, 1:4, :], x_r[:, 1:4, :])

            def wload(dram, shape):
                t = const.tile(list(shape), bf, tag=dram.name)
                nc.sync.dma_start(t[:], dram[:])
                return t

            nc.sync.dma_start(aff[:], aff_d[:])
            wv = wload(wv_d, (128, 2, D))
            wqkv = wload(wqkv_d, (128, 2, 3 * D))
            for qg_ in range(1, 4):
                nc.sync.dma_start(x_all[:, qg_ * 4:(qg_ + 1) * 4, :],
                                  x_r[:, qg_ * 4:(qg_ + 1) * 4, :])
            wk = wload(wk_d, (128, 2, D))
            wq = wload(wq_d, (128, 2, D))
            lt_r = lt_d.rearrange("(n p) t -> p n t", p=128)
            lt_all = pers.tile([128, NT, SH], bf)
            for qg_ in range(4):
                nc.sync.dma_start(lt_all[:, qg_ * 4:(qg_ + 1) * 4, :],
                                  lt_r[:, qg_ * 4:(qg_ + 1) * 4, :])
            wf1 = wload(wf1_d, (128, 2, D))
            wf2 = wload(wf2_d, (128, 2, 2))
            wg1 = wload(wg1_d, (128, 4, D))
            wg2 = wload(wg2_d, (128, 2, D))
            wgo = wload(wgo_d, (128, 2, D))
            wff1 = wload(wff1_d, (128, 2, 4 * D))
            wff2 = wload(wff2_d, (128, 8, D))

            hT = pers.tile([128, 2, L], bf)    # h^T, full batch
            hKT = pers.tile([128, 2, L], bf)   # (h@Wk)^T, full batch
            hV = pers.tile([128, NT, D], bf)   # h@Wv@Wlo, token-major
            tap_tiles["hT"], tap_tiles["hKT"], tap_tiles["hV"] = hT, hKT, hV
            qT = pers.tile([128, 2, SH], bf)
            tap_tiles["qT"] = qT

            f1T = pers.tile([128, 2, SH], bf)
            qg_all = pers.tile([128, ST, D], f32)
            kv_ps = [psacc.tile([128, 257], f32, tag="acc", name=f"kv{g}")
                     for g in range(2)]
            agg_ps = [psacc.tile([128, 512], f32, tag="acc", name=f"agg{g}")
                      for g in range(2)]
            den_acc = pssml.tile([1, 512], f32, tag="accs", name="den")

            rstd_rest = pers.tile([128, 12], f32)
            nmr_rest = pers.tile([128, 12], f32)

            heron_n = [0]

            def heron_core(rstd_out, nmr_out, mean_ap, var_ap, iters, k):
                """rstd = 1/sqrt(var + 1e-5), nmr = -mean*rstd.  All-DVE
                Heron iterations (no ACT sqrt table)."""
                heron_n[0] += 1
                hid = heron_n[0]
                vh = stm.tile([128, k], f32, tag="her", name=f"vh_{hid}")
                nc.vector.tensor_scalar(vh[:], var_ap, 0.5, 5e-6,
                                        op0=ALU.mult, op1=ALU.add)
                s = stm.tile([128, k], f32, tag="her", name=f"s_{hid}")
                nc.vector.tensor_scalar_add(s[:], vh[:], 0.5)
                r = stm.tile([128, k], f32, tag="her", name=f"r_{hid}")
                q = stm.tile([128, k], f32, tag="her", name=f"q_{hid}")
                for _ in range(iters):
                    nc.vector.reciprocal(r[:], s[:])
                    nc.vector.tensor_mul(q[:], vh[:], r[:])
                    nc.vector.scalar_tensor_tensor(s[:], s[:], 0.5, q[:],
                                                   op0=ALU.mult, op1=ALU.add)
                nc.vector.reciprocal(rstd_out, s[:])
                nc.vector.scalar_tensor_tensor(nmr_out, mean_ap, -1.0,
                                               rstd_out, op0=ALU.mult,
                                               op1=ALU.mult)

            def heron_rstd(rstd_out, nmr_out, mean_c, msq_c, iters):
                k = mean_c.shape[-1]
                hid = heron_n[0] + 100
                m2 = stm.tile([128, k], f32, tag="her", name=f"m2_{hid}")
                nc.vector.tensor_mul(m2[:], mean_c[:], mean_c[:])
                df = stm.tile([128, k], f32, tag="her", name=f"df_{hid}")
                nc.vector.tensor_sub(df[:], msq_c[:], m2[:])
                heron_core(rstd_out, nmr_out, mean_c[:], df[:], iters, k)

            # producers for the software-pipelined accumulators
            kg_tiles = {}
            ut_tiles = {}

            def emit_kv(n):
                kg_l, vg_rhs = kg_tiles.pop(n)
                for g in range(2):
                    nc.tensor.matmul(kv_ps[g][:], kg_l[:, g * 128:(g + 1) * 128],
                                     vg_rhs[:], start=(n == 0), stop=(n == NT - 1))

            def emit_attn_acc(jc):
                ut = ut_tiles.pop(jc)
                for g in range(2):
                    nc.tensor.matmul(agg_ps[g][:], hV[:, jc, g * 128:(g + 1) * 128],
                                     ut[:], start=(jc == 0), stop=(jc == NT - 1))

            # ---------- fused pre-work + attention, per 4-tile group ----------
            for qgrp in range(4):
                tiles = range(qgrp * 4, qgrp * 4 + 4)
                if qgrp == 0:
                    mval = stm.tile([128, 4, 2], f32, tag="mval")
                    rstd4 = stm.tile([128, 4], f32, tag="rstd4")
                    nmr4 = stm.tile([128, 4], f32, tag="nmr4")
                    stats = stm.tile([128, 6], f32, tag="stats")
                    nc.vector.bn_stats(out=stats[:], in_=x_all[:, 0, :])
                    nc.vector.bn_aggr(out=mval[:, 0, :], in_=stats[:])
                    heron_core(rstd4[:, 0:1], nmr4[:, 0:1], mval[:, 0, 0:1],
                               mval[:, 0, 1:2], iters=3, k=1)
                else:
                    rstd4 = rstd_rest[:, (qgrp - 1) * 4:qgrp * 4]
                    nmr4 = nmr_rest[:, (qgrp - 1) * 4:qgrp * 4]
                for i, n in enumerate(tiles):
                    if qgrp == 0 and i == 1:
                        # stats for tiles 1-3 emitted after tile 0's chain so
                        # the first transpose isn't queued behind them
                        for i2 in range(1, 4):
                            stats = stm.tile([128, 6], f32, tag="stats")
                            nc.vector.bn_stats(out=stats[:], in_=x_all[:, i2, :])
                            nc.vector.bn_aggr(out=mval[:, i2, :], in_=stats[:])
                        heron_core(rstd4[:, 1:4], nmr4[:, 1:4], mval[:, 1:4, 0],
                                   mval[:, 1:4, 1], iters=3, k=3)
                    js = slice(n * 128, (n + 1) * 128)
                    # hn = (x - m) * rstd  (one fused DVE op, bf16 out)
                    hn = hnp.tile([128, D], f32, tag="hn")
                    nc.vector.tensor_scalar(hn[:], x_all[:, n, :],
                                            rstd4[:, i:i + 1], nmr4[:, i:i + 1],
                                            op0=ALU.mult, op1=ALU.add)
                    for c in range(2):
                        pt = psmm.tile([128, 128], f32, tag="mm")
                        nc.tensor.transpose(pt[:], hn[:, c * 128:(c + 1) * 128],
                                            ident[:])
                        # h = hn * g1 + b1 on the transposed copy-out (DVE)
                        nc.vector.tensor_scalar(hT[:, c, js], pt[:],
                                                aff[:, c, 0:1], aff[:, c, 1:2],
                                                op0=ALU.mult, op1=ALU.add)
                    # hV tile
                    pv = psmm.tile([128, D], f32, tag="mm")
                    for c in range(2):
                        nc.tensor.matmul(pv[:], hT[:, c, js], wv[:, c, :],
                                         start=(c == 0), stop=(c == 1))
                    nc.scalar.copy(hV[:, n, :], pv[:])
                    # kg/vg projection + elu(k)+1 = min(exp(k),1) + max(k,0)
                    pq = psmm.tile([128, 512], f32, tag="mm")
                    for c in range(2):
                        nc.tensor.matmul(pq[:], hT[:, c, js], wqkv[:, c, D:3 * D],
                                         start=(c == 0), stop=(c == 1))
                    te = stmq.tile([128, D], bf, tag="tmpq")
                    nc.scalar.activation(te[:], pq[:, 0:D], AF.Exp)
                    ta_ = stmq.tile([128, D], bf, tag="tmpq")
                    nc.vector.tensor_scalar_min(ta_[:], te[:], 1.0)
                    tr = stmq.tile([128, D], bf, tag="tmpq")
                    nc.vector.tensor_scalar_max(tr[:], pq[:, 0:D], 0.0)
                    kg_l = stm.tile([128, D], bf, tag="kg_l")
                    nc.vector.tensor_add(kg_l[:], ta_[:], tr[:])
                    vg_rhs = stm.tile([128, D + 1], bf, tag="vg_rhs")
                    nc.vector.tensor_copy(vg_rhs[:, 0:D], pq[:, D:2 * D])
                    nc.gpsimd.memset(vg_rhs[:, D:D + 1], 1.0)
                    kg_tiles[n] = (kg_l, vg_rhs)
                    if n >= 2:
                        emit_kv(n - 2)

                # hKT chunk for this group
                jsg = slice(qgrp * 512, (qgrp + 1) * 512)
                for g in range(2):
                    pk = psmm.tile([128, 512], f32, tag="mm")
                    for c in range(2):
                        nc.tensor.matmul(pk[:], wk[:, c, g * 128:(g + 1) * 128],
                                         hT[:, c, jsg], start=(c == 0), stop=(c == 1))
                    nc.scalar.copy(hKT[:, g, jsg], pk[:])
                # qT + linear-attn qg (needs hT tiles 0..3 only)
                if qgrp == 0:
                    for g in range(2):
                        pq2 = psmm.tile([128, 512], f32, tag="mm")
                        for c in range(2):
                            nc.tensor.matmul(pq2[:], wq[:, c, g * 128:(g + 1) * 128],
                                             hT[:, c, 0:SH], start=(c == 0), stop=(c == 1))
                        nc.vector.tensor_copy(qT[:, g, :], pq2[:])
                    for it in range(ST):
                        ts_ = slice(it * 128, (it + 1) * 128)
                        pq3 = psmm.tile([128, D], f32, tag="mm")
                        for c in range(2):
                            nc.tensor.matmul(pq3[:], hT[:, c, ts_], wqkv[:, c, 0:D],
                                             start=(c == 0), stop=(c == 1))
                        teb = stmq.tile([128, D], f32, tag="tmpq")
                        nc.scalar.activation(teb[:], pq3[:], AF.Exp)
                        tab_ = stmq.tile([128, D], f32, tag="tmpq")
                        nc.vector.tensor_scalar_min(tab_[:], teb[:], 1.0)
                        trb = stmq.tile([128, D], f32, tag="tmpq")
                        nc.vector.tensor_scalar_max(trb[:], pq3[:], 0.0)
                        nc.vector.tensor_add(qg_all[:, it, :], tab_[:], trb[:])
                    # batched LN stats for tiles 4..15 (DVE bn + Heron)
                    mv_r = stm.tile([128, 12, 2], f32, tag="mv_r")
                    for i2, n2 in enumerate(range(4, NT)):
                        stats2 = stm.tile([128, 6], f32, tag="stats")
                        nc.vector.bn_stats(out=stats2[:], in_=x_all[:, n2, :])
                        nc.vector.bn_aggr(out=mv_r[:, i2, :], in_=stats2[:])
                    heron_core(rstd_rest[:], nmr_rest[:], mv_r[:, :, 0],
                               mv_r[:, :, 1], iters=3, k=12)

                # attention chunks for this group (acc pipelined one behind)
                for jc in tiles:
                    js = slice(jc * 128, (jc + 1) * 128)
                    pl = psmm.tile([128, 512], f32, tag="mm")
                    nc.tensor.matmul(pl[:], hKT[:, 0, js], qT[:, 0, :],
                                     start=True, stop=False)
                    nc.tensor.matmul(pl[:], hKT[:, 1, js], qT[:, 1, :],
                                     start=False, stop=False)
                    nc.tensor.matmul(pl[:], ident_bf[:], lt_all[:, jc, :],
                                     start=False, stop=True)
                    ut = stm.tile([128, 512], bf, tag="ut")
                    nc.scalar.activation(ut[:], pl[:], AF.Exp)
                    nc.tensor.matmul(den_acc[:], ones_cb[:], ut[:],
                                     start=(jc == 0), stop=(jc == NT - 1))
                    ut_tiles[jc] = ut
                    if jc >= 2:
                        emit_attn_acc(jc - 2)
            emit_kv(NT - 2)
            emit_kv(NT - 1)
            emit_attn_acc(NT - 2)
            emit_attn_acc(NT - 1)

            # ---------- tail: reordered for cross-engine overlap -------------
            aggloT = pers.tile([128, 2, SH], bf)
            tap_tiles["aggloT"] = aggloT
            g1T = pers.tile([128, 2, SH], bf)
            tgate = pers.tile([128, 2, SH], bf)
            tap_tiles["tgate"] = tgate
            h_localT = pers.tile([128, 2, SH], f32)
            tap_tiles["h_localT"] = h_localT
            qgzT = pers.tile([128, 2, SH], bf)
            tap_tiles["qgzT"] = qgzT
            yT = pers.tile([128, 2, SH], bf)
            tap_tiles["yT"] = yT
            h_globalT = pers.tile([128, 2, SH], f32)
            tap_tiles["h_globalT"] = h_globalT
            xoT = pers.tile([128, 2, SH], f32)
            tap_tiles["xoT"] = xoT
            xo_bf = pers.tile([128, 2, SH], bf)
            xnT = pers.tile([128, 2, SH], bf)
            tap_tiles["xnT"] = xnT
            ff1T = pers.tile([128, 8, SH], bf)
            outT = pers.tile([128, 2, SH], f32)
            tap_tiles["outT"] = outT

            # kv block-diagonal + ksum extraction (ACT/DVE, first thing)
            kvb = pers.tile([128, 2, D], bf)
            tap_tiles["kvb"] = kvb
            nc.vector.memset(kvb[:], 0.0)
            for h in range(H):
                g, po = h // 4, (h * DH) % 128
                nc.scalar.copy(kvb[po:po + DH, g, h * DH:(h + 1) * DH],
                               kv_ps[g][po:po + DH, h * DH:(h + 1) * DH])
            ksum_col = pers.tile([128, 2], f32)
            for g in range(2):
                nc.vector.tensor_copy(ksum_col[:, g:g + 1], kv_ps[g][:, D:D + 1])
            ksum_row = pers.tile([1, D], bf)
            for g in range(2):
                pt = psmm.tile([128, 128], f32, tag="mm")
                nc.tensor.transpose(pt[0:1, 0:128], ksum_col[:, g:g + 1], ident[:])
                nc.vector.tensor_copy(ksum_row[0:1, g * 128:(g + 1) * 128],
                                      pt[0:1, 0:128])
            kb_ps = psmm.tile([128, D], f32, tag="mm")
            nc.tensor.matmul(kb_ps[:], ones_rb[:], ksum_row[:], start=True, stop=True)
            ksumb = pers.tile([128, D], bf)
            tap_tiles["ksumb"] = ksumb
            nc.vector.tensor_copy(ksumb[:], kb_ps[:])

            # gate first-level matmuls on h (independent of agglo) keep PE busy
            pgs = [psacc.tile([128, 512], f32, tag="acc", name=f"pg{g}")
                   for g in range(2)]
            for g in range(2):
                for c in range(2):
                    nc.tensor.matmul(pgs[g][:], wg1[:, c, g * 128:(g + 1) * 128],
                                     hT[:, c, 0:SH], start=(c == 0), stop=False)

            # denominator reciprocal + agglo
            den_sb2 = stm.tile([1, 512], f32, tag="den_sb2")
            nc.vector.tensor_copy(den_sb2[:], den_acc[:])
            den_rb = pers.tile([1, 512], bf)
            tap_tiles["den_rb"] = den_rb
            with nc.allow_low_precision("bf16 recip feeds bf16 broadcast"):
                nc.vector.reciprocal(den_rb[:], den_sb2[:])
            rbp = psmm.tile([128, 512], f32, tag="mm", name="rbp")
            nc.tensor.matmul(rbp[:], ones_rb[:], den_rb[:], start=True, stop=True)
            rbh = stmf.tile([128, 512], bf, tag="tmpf", name="rbh")
            nc.scalar.copy(rbh[:], rbp[:])
            for g in range(2):
                nc.vector.tensor_mul(aggloT[:, g, :], agg_ps[g][:], rbh[:])

            # f1 chain matmuls + first gelu (triggers the one gelu table load)
            for g in range(2):
                pf = psmm.tile([128, 512], f32, tag="mm")
                for c in range(2):
                    nc.tensor.matmul(pf[:], wf1[:, c, g * 128:(g + 1) * 128],
                                     hT[:, c, 0:SH], start=(c == 0), stop=(c == 1))
                nc.scalar.activation(f1T[:, g, :], pf[:], AF.Gelu)

            # gate second half (agglo) + g1 gelu
            for g in range(2):
                for c in range(2):
                    nc.tensor.matmul(pgs[g][:], wg1[:, 2 + c, g * 128:(g + 1) * 128],
                                     aggloT[:, c, :], start=False, stop=(c == 1))
                nc.scalar.activation(g1T[:, g, :], pgs[g][:], AF.Gelu)

            # linear attention z + qgz (DVE) overlapping the gate/f1 chains
            zden_a = stm.tile([128, ST, H], f32, tag="zden_a")
            for it in range(ST):
                prod = stmq.tile([128, D], f32, tag="tmpq")
                nc.vector.tensor_mul(prod[:], qg_all[:, it, :], ksumb[:])
                nc.vector.tensor_reduce(zden_a[:, it, :],
                                        prod[:].rearrange("p (h d) -> p h d", d=DH),
                                        axis=AX.X, op=ALU.add)
            nc.vector.tensor_scalar_add(zden_a[:], zden_a[:], 1e-6)
            zr_a = stm.tile([128, ST, H], f32, tag="zr_a")
            nc.vector.reciprocal(
                zr_a[:].rearrange("p a b -> p (a b)"),
                zden_a[:].rearrange("p a b -> p (a b)"))
            for it in range(ST):
                ts_ = slice(it * 128, (it + 1) * 128)
                qgz = stmq.tile([128, D], f32, tag="tmpq")
                nc.vector.tensor_tensor(
                    out=qgz[:].rearrange("p (h d) -> p h d", d=DH),
                    in0=qg_all[:, it, :].rearrange("p (h d) -> p h d", d=DH),
                    in1=zr_a[:, it, :].to_broadcast([128, H, DH]), op=ALU.mult)
                for c in range(2):
                    ptq = psmm.tile([128, 128], f32, tag="mm")
                    nc.tensor.transpose(ptq[:], qgz[:, c * 128:(c + 1) * 128],
                                        ident[:])
                    nc.vector.tensor_copy(qgzT[:, c, ts_], ptq[:])

            # wf fuse-gate weights
            wf_ps = pssml.tile([2, 512], f32, tag="accs", name="wfps")
            wf_sb = stm.tile([2, 512], bf, tag="wf_sb")
            for c in range(2):
                nc.tensor.matmul(wf_ps[:], wf2[:, c, :], f1T[:, c, :],
                                 start=(c == 0), stop=(c == 1))
            nc.scalar.copy(wf_sb[:], wf_ps[:])
            d01_ps = psmm.tile([1, 512], f32, tag="mm", name="d01")
            nc.tensor.matmul(d01_ps[:], pm[:], wf_sb[:], start=True, stop=True)
            th_wf = stm.tile([1, 512], bf, tag="th_wf")
            nc.scalar.activation(th_wf[:], d01_ps[:], AF.Tanh, scale=0.5)
            wf0 = pers.tile([1, 512], bf)
            tap_tiles["wf0"] = wf0
            wf1s = pers.tile([1, 512], bf)
            nc.vector.tensor_scalar(wf0[:], th_wf[:], 0.5, 0.5,
                                    op0=ALU.mult, op1=ALU.add)
            nc.vector.tensor_scalar(wf1s[:], th_wf[:], -0.5, 0.5,
                                    op0=ALU.mult, op1=ALU.add)

            # gate second level + tanh; y and h_global in parallel
            for g in range(2):
                pg2 = psmm.tile([128, 512], f32, tag="mm")
                for c in range(2):
                    nc.tensor.matmul(pg2[:], wg2[:, c, g * 128:(g + 1) * 128],
                                     g1T[:, c, :], start=(c == 0), stop=(c == 1))
                nc.scalar.activation(tgate[:, g, :], pg2[:], AF.Tanh, scale=0.5)
            for g in range(2):
                py = psmm.tile([128, 512], f32, tag="mm")
                for c in range(2):
                    nc.tensor.matmul(py[:], kvb[:, c, g * 128:(g + 1) * 128],
                                     qgzT[:, c, :], start=(c == 0), stop=(c == 1))
                nc.vector.tensor_copy(yT[:, g, :], py[:])
            for g in range(2):
                pgo = psmm.tile([128, 512], f32, tag="mm")
                for c in range(2):
                    nc.tensor.matmul(pgo[:], wgo[:, c, g * 128:(g + 1) * 128],
                                     yT[:, c, :], start=(c == 0), stop=(c == 1))
                nc.vector.tensor_add(h_globalT[:, g, :], hT[:, g, 0:SH], pgo[:])
            # h_local = h + sigmoid(gate)*agglo = h + 0.5*(agglo + agglo*tanh)
            for g in range(2):
                u = stmf.tile([128, 512], bf, tag="tmpf")
                nc.gpsimd.tensor_mul(u[:], tgate[:, g, :], aggloT[:, g, :])
                v = stmf.tile([128, 512], bf, tag="tmpf")
                nc.gpsimd.tensor_add(v[:], aggloT[:, g, :], u[:])
                nc.vector.scalar_tensor_tensor(h_localT[:, g, :], v[:], 0.5,
                                               hT[:, g, 0:SH],
                                               op0=ALU.mult, op1=ALU.add)

            # xo = wf0*h_local + wf1*h_global
            b0p = psmm.tile([128, 512], f32, tag="mm", name="b0p")
            nc.tensor.matmul(b0p[:], ones_rb[:], wf0[:], start=True, stop=True)
            b1p = psmm.tile([128, 512], f32, tag="mm", name="b1p")
            nc.tensor.matmul(b1p[:], ones_rb[:], wf1s[:], start=True, stop=True)
            for g in range(2):
                ta = stmf.tile([128, 512], f32, tag="tmpf")
                nc.vector.tensor_mul(ta[:], h_localT[:, g, :], b0p[:])
                tb = stmf.tile([128, 512], f32, tag="tmpf")
                nc.vector.tensor_mul(tb[:], h_globalT[:, g, :], b1p[:])
                nc.vector.tensor_add(xoT[:, g, :], ta[:], tb[:])
                nc.vector.tensor_copy(xo_bf[:, g, :], xoT[:, g, :])

            # ---------- LN2: stats via matmul, rsqrt via column Heron ---------
            sum_ps = pssml.tile([1, 512], f32, tag="accs", name="sumps")
            for c in range(2):
                nc.tensor.matmul(sum_ps[:], ones_cb[:], xo_bf[:, c, :],
                                 start=(c == 0), stop=(c == 1))
            ssq_ps = psmm.tile([1, 512], f32, tag="mm", name="ssqps")
            for c in range(2):
                xsq = stmf.tile([128, 512], bf, tag="xsq")
                nc.scalar.activation(xsq[:], xoT[:, c, :], AF.Square)
                nc.tensor.matmul(ssq_ps[:], ones_cb[:], xsq[:],
                                 start=(c == 0), stop=(c == 1))
            sum_row = stm.tile([1, 512], f32, tag="sum_row")
            nc.vector.tensor_copy(sum_row[:], sum_ps[:])
            ssq_row = stm.tile([1, 512], f32, tag="ssq_row")
            nc.vector.tensor_copy(ssq_row[:], ssq_ps[:])
            sq_col = stm.tile([128, 4, 2], f32, tag="sq_col")
            for kk in range(4):
                pts = psmm.tile([128, 128], f32, tag="mm")
                nc.tensor.transpose(pts[0:128, 0:1],
                                    sum_row[0:1, kk * 128:(kk + 1) * 128],
                                    ident[0:1, 0:1])
                nc.vector.tensor_copy(sq_col[:, kk, 0:1], pts[0:128, 0:1])
                ptq2 = psmm.tile([128, 128], f32, tag="mm")
                nc.tensor.transpose(ptq2[0:128, 0:1],
                                    ssq_row[0:1, kk * 128:(kk + 1) * 128],
                                    ident[0:1, 0:1])
                nc.vector.tensor_copy(sq_col[:, kk, 1:2], ptq2[0:128, 0:1])
            mean_c = stm.tile([128, 4], f32, tag="mean_c")
            nc.vector.tensor_scalar_mul(mean_c[:], sq_col[:, :, 0], 1.0 / D)
            msq_c = stm.tile([128, 4], f32, tag="msq_c")
            nc.vector.tensor_scalar_mul(msq_c[:], sq_col[:, :, 1], 1.0 / D)
            rstd_c = stm.tile([128, 4], f32, tag="rstd_c")
            nmr_c = stm.tile([128, 4], f32, tag="nmr_c")
            heron_rstd(rstd_c[:], nmr_c[:], mean_c, msq_c, iters=5)
            nmr_row = stm.tile([1, 512], bf, tag="nmr_row")
            rstd_row = stm.tile([1, 512], bf, tag="rstd_row")
            for kk in range(4):
                ptb0 = psmm.tile([128, 128], f32, tag="mm")
                nc.tensor.transpose(ptb0[0:1, 0:128], nmr_c[:, kk:kk + 1],
                                    ident[:])
                nc.vector.tensor_copy(nmr_row[:, kk * 128:(kk + 1) * 128],
                                      ptb0[0:1, 0:128])
                ptb1 = psmm.tile([128, 128], f32, tag="mm")
                nc.tensor.transpose(ptb1[0:1, 0:128], rstd_c[:, kk:kk + 1],
                                    ident[:])
                nc.vector.tensor_copy(rstd_row[:, kk * 128:(kk + 1) * 128],
                                      ptb1[0:1, 0:128])
            nmr_b = psmm.tile([128, 512], f32, tag="mm", name="nmrb")
            nc.tensor.matmul(nmr_b[:], ones_rb[:], nmr_row[:],
                             start=True, stop=True)
            rb2_b = psmm.tile([128, 512], f32, tag="mm", name="rb2b")
            nc.tensor.matmul(rb2_b[:], ones_rb[:], rstd_row[:],
                             start=True, stop=True)
            for g in range(2):
                t1 = stmf.tile([128, 512], f32, tag="tmpf")
                nc.vector.tensor_mul(t1[:], xoT[:, g, :], rb2_b[:])
                # g2 is folded into Wff1 on the host; write bf16 directly
                nc.vector.tensor_add(xnT[:, g, :], t1[:], nmr_b[:])

            # FFN + residual + output transposes/stores
            pf2s = [psacc.tile([128, 512], f32, tag="acc", name=f"pf2_{g}")
                    for g in range(2)]
            for g8 in range(8):
                pff = psmm.tile([128, 512], f32, tag="mm")
                for c in range(2):
                    nc.tensor.matmul(pff[:], wff1[:, c, g8 * 128:(g8 + 1) * 128],
                                     xnT[:, c, :], start=(c == 0), stop=(c == 1))
                nc.scalar.activation(ff1T[:, g8, :], pff[:], AF.Gelu)
                if g8 >= 1:
                    for g in range(2):
                        nc.tensor.matmul(pf2s[g][:],
                                         wff2[:, g8 - 1, g * 128:(g + 1) * 128],
                                         ff1T[:, g8 - 1, :],
                                         start=(g8 == 1), stop=False)
            for g in range(2):
                nc.tensor.matmul(pf2s[g][:], wff2[:, 7, g * 128:(g + 1) * 128],
                                 ff1T[:, 7, :], start=False, stop=True)
                nc.vector.tensor_add(outT[:, g, :], xoT[:, g, :], pf2s[g][:])
            for it in range(ST):
                ts_ = slice(it * 128, (it + 1) * 128)
                ot = stmq.tile([128, D], f32, tag="tmpq")
                for c in range(2):
                    pto = psmm.tile([128, 128], f32, tag="mm")
                    nc.tensor.transpose(pto[:], outT[:, c, ts_], ident[:])
                    nc.vector.tensor_copy(ot[:, c * 128:(c + 1) * 128], pto[:])
                nc.sync.dma_start(out_d[ts_, :], ot[:])

            for name in taps:
                t = tap_tiles[name]
                td = nc.dram_tensor(f"tap_{name}", list(t.shape),
                                    t.dtype, kind="ExternalOutput")
                nc.sync.dma_start(td[:], t[:])

    nc.compile()
    return nc


def _host_prep(inputs):
    """Host-side preprocessing shared by all cores + per-core arrays."""
    x = np.asarray(inputs["x"], np.float32)
    mask = np.asarray(inputs["mask"])
    nbr_idx = np.asarray(inputs["nbr_idx"]).astype(np.int64)
    nbr_mask = np.asarray(inputs["nbr_mask"])
    rel_pos = np.asarray(inputs["rel_pos"]).astype(np.int64)

    if not (np.all(mask == 1)):
        raise NotImplementedError("kernel assumes mask == ones (spec fill)")

    # edge-bias table over the 65 possible rel values
    Erel = np.asarray(inputs["Erel"], np.float32)
    We1 = np.asarray(inputs["We1"], np.float32)
    be1 = np.asarray(inputs["be1"], np.float32)
    We2 = np.asarray(inputs["We2"], np.float32)
    be2 = np.asarray(inputs["be2"], np.float32)
    tab = (_gelu_np(Erel @ We1 + be1) @ We2 + be2)[:, 0]  # [65]

    rel = np.clip(rel_pos, -CLIP, CLIP) + CLIP
    ev = np.exp(tab[rel]) * (nbr_mask != 0)  # [B, L, K]

    # dense E^T per batch: ET[b][j, t] = sum_k ev[b,t,k] * [idx==j]
    ET = np.zeros((B, L, L), np.float32)
    for b in range(B):
        t_idx = np.repeat(np.arange(L), K)
        np.add.at(ET[b], (nbr_idx[b].ravel(), t_idx), ev[b].ravel())
    # log-domain (folded into the score PSUM in-kernel): -1e30 where empty
    LT = np.where(ET > 0, np.log(np.maximum(ET, 1e-30)), -1e30).astype(np.float32)

    aff = np.zeros((128, 2, 4), np.float32)
    for name, i in (("g1", 0), ("b1", 1), ("g2", 2), ("b2", 3)):
        v = np.asarray(inputs[name], np.float32)
        aff[:, :, i] = v.reshape(2, 128).T

    shared = {
        "aff": aff,
        "wq": _w_tiles(np.asarray(inputs["Wq"], np.float32) / 16.0, 2),
        "wk": _w_tiles(np.asarray(inputs["Wk"], np.float32), 2),
        "wv": _w_tiles(np.asarray(inputs["Wv"], np.float32)
                       @ np.asarray(inputs["Wlo"], np.float32), 2),
        "wg1": _w_tiles(np.asarray(inputs["Wg1"], np.float32), 4),
        "wg2": _w_tiles(np.asarray(inputs["Wg2"], np.float32), 2),
        "wqkv": _w_tiles(np.asarray(inputs["Wqkv"], np.float32), 2),
        "wgo": _w_tiles(np.asarray(inputs["Wgo"], np.float32), 2),
        "wf1": _w_tiles(np.asarray(inputs["Wf1"], np.float32), 2),
        "wf2": _w_tiles(np.asarray(inputs["Wf2"], np.float32), 2),
        "wff1": _w_tiles(np.asarray(inputs["g2"], np.float32)[:, None]
                         * np.asarray(inputs["Wff1"], np.float32), 2),
        "wff2": _w_tiles(np.asarray(inputs["Wff2"], np.float32), 8),
        "pm": np.array([[1.0], [-1.0]], BF16),
    }
    for k in ("blo", "bg1", "bg2", "bf1", "bf2", "bff1", "bff2", "b2"):
        if not np.allclose(np.asarray(inputs[k]), 0.0):
            raise NotImplementedError(f"kernel assumes bias {k} == 0 (spec fill)")

    per_core = []
    for c in range(NCORES):
        b, s = c // SPB, c % SPB
        s0 = s * SH
        xp = np.roll(x[b], -s0, axis=0)
        ltp = np.roll(LT[b][:, s0:s0 + SH], -s0, axis=0).astype(BF16)
        per_core.append({"x": np.ascontiguousarray(xp),
                         "lt": np.ascontiguousarray(ltp)})
    return shared, per_core


def kernel(**inputs) -> np.ndarray:
    import concourse.bass_utils as bu

    if "nc" not in _CACHE:
        _CACHE["nc"] = _build()
    nc = _CACHE["nc"]

    shared, per_core = _host_prep(inputs)
    in_maps = [{**shared, **pc} for pc in per_core]
    res = bu.run_bass_kernel_spmd(nc, in_maps, core_ids=list(range(NCORES)))
    out = np.zeros((B, L, D), np.float32)
    for c in range(NCORES):
        b, s = c // SPB, c % SPB
        out[b, s * SH:(s + 1) * SH] = res.results[c]["out"]
    return out


# revision 25
# speedup vs baseline: 1.1619x; 1.0199x over previous
"""Trainium2 Bass kernel for nn_Druggability_DistillModel (gnn_message_passing).

Strategy (8 NeuronCores, data-parallel over B x 4-way sequence shards):
  - core c handles batch b=c//4, tokens [s*512, (s+1)*512) with s=c%4.
  - The edge-bias MLP depends only on rel_pos (65 values) -> host collapses
    it to a table and builds LT[j, t] = log(sum_dup exp(edge)) over neighbors
    (−1e30 where none), so softmax_k(q.k/16 + edge) * v becomes
    exp(q.hK^T + LT) @ hV / rowsum — dense PE work, no gather.
  - LT is folded into the score PSUM via an identity matmul, so the sweep is
    matmul→matmul→matmul→Exp with no elementwise hop in between.
  - Denominators accumulate as rows of one [16,512] PSUM tile (one matmul per
    j-tile) and reduce with a single ones^T matmul at the end.
  - ACT table discipline: the scalar engine only ever loads the exp set (up
    front, via a dummy op that overlaps the first DMAs) and the gelu set (for
    the tail: gelu + tanh-as-sigmoid + square).  All rsqrt work (both
    layernorms) runs on the DVE as Heron iterations seeded from (1+v)/2.
  - PE warm-up: a burst of identity matmuls at t~0 lifts the HAM clock gate
    to full rate before the real prework arrives.
"""
import sys

sys.path.insert(0, "/opt/trn_rl_repo")

import math
import numpy as np
import ml_dtypes

B, L, D, H, DH, K, DE, CLIP = 2, 2048, 256, 8, 32, 36, 64, 32
NCORES, SPB, SH = 8, 4, 512  # cores, shards/batch, tokens/shard
NT = L // 128                # 16 token tiles per batch
ST = SH // 128               # 4 tiles per shard
BF16 = ml_dtypes.bfloat16

_CACHE: dict = {}


def _gelu_np(x):
    try:
        from scipy.special import erf
        e = erf(x / np.sqrt(2.0))
    except Exception:
        import math as _m
        e = np.vectorize(_m.erf)(x / np.sqrt(2.0))
    return x * 0.5 * (1.0 + e)


def _w_tiles(w, cin_chunks):
    """[din, dout] -> [128, cin_chunks, dout] with din = c*128+p."""
    din, dout = w.shape
    assert din == cin_chunks * 128
    return np.ascontiguousarray(
        w.reshape(cin_chunks, 128, dout).transpose(1, 0, 2)
    ).astype(BF16)


def _build(taps=()):
    import concourse.bass as bass
    import concourse.tile as tile
    from concourse import bacc, mybir
    from concourse.masks import make_identity

    f32, bf = mybir.dt.float32, mybir.dt.bfloat16
    AF = mybir.ActivationFunctionType
    ALU = mybir.AluOpType
    AX = mybir.AxisListType

    nc = bacc.Bacc("TRN2", target_bir_lowering=False, debug=False)

    x_d = nc.dram_tensor("x", [L, D], f32, kind="ExternalInput")
    lt_d = nc.dram_tensor("lt", [L, SH], bf, kind="ExternalInput")
    aff_d = nc.dram_tensor("aff", [128, 2, 4], f32, kind="ExternalInput")
    wq_d = nc.dram_tensor("wq", [128, 2, D], bf, kind="ExternalInput")
    wk_d = nc.dram_tensor("wk", [128, 2, D], bf, kind="ExternalInput")
    wv_d = nc.dram_tensor("wv", [128, 2, D], bf, kind="ExternalInput")
    wg1_d = nc.dram_tensor("wg1", [128, 4, D], bf, kind="ExternalInput")
    wg2_d = nc.dram_tensor("wg2", [128, 2, D], bf, kind="ExternalInput")
    wqkv_d = nc.dram_tensor("wqkv", [128, 2, 3 * D], bf, kind="ExternalInput")
    wgo_d = nc.dram_tensor("wgo", [128, 2, D], bf, kind="ExternalInput")
    wf1_d = nc.dram_tensor("wf1", [128, 2, D], bf, kind="ExternalInput")
    wf2_d = nc.dram_tensor("wf2", [128, 2, 2], bf, kind="ExternalInput")
    wff1_d = nc.dram_tensor("wff1", [128, 2, 4 * D], bf, kind="ExternalInput")
    wff2_d = nc.dram_tensor("wff2", [128, 8, D], bf, kind="ExternalInput")
    pm_d = nc.dram_tensor("pm", [2, 1], bf, kind="ExternalInput")
    out_d = nc.dram_tensor("out", [SH, D], f32, kind="ExternalOutput")
    tap_tiles = {}

    with tile.TileContext(nc) as tc:
        with (
            tc.tile_pool(name="const", bufs=1) as const,
            tc.tile_pool(name="persist", bufs=1) as pers,
            tc.tile_pool(name="stream", bufs=4) as stm,
            tc.tile_pool(name="stmf", bufs=3) as stmf,
            tc.tile_pool(name="stmq", bufs=8) as stmq,
            tc.tile_pool(name="hnp", bufs=3) as hnp,
            tc.tile_pool(name="psmm", bufs=3, space="PSUM") as psmm,
            tc.tile_pool(name="psacc", bufs=4, space="PSUM") as psacc,
            tc.tile_pool(name="pssml", bufs=1, space="PSUM") as pssml,
        ):
            ident = const.tile([128, 128], f32)
            make_identity(nc, ident[:])
            ident_bf = const.tile([128, 128], bf)
            make_identity(nc, ident_bf[:])
            ones_cb = const.tile([128, 1], bf)
            nc.vector.memset(ones_cb[:], 1.0)
            ones_rb = const.tile([1, 128], bf)
            nc.vector.memset(ones_rb[:], 1.0)
            pm = const.tile([2, 1], bf)
            nc.sync.dma_start(pm[:], pm_d[:])
            eps5 = const.tile([128, 1], f32)
            nc.vector.memset(eps5[:], 1e-5)
            ones_f1 = const.tile([1, 1], f32)
            nc.vector.memset(ones_f1[:], 1.0)
            aff = const.tile([128, 2, 4], f32)

            # preload the EXP activation table while DMAs stream in
            scr_e = const.tile([1, 1], f32)
            nc.scalar.activation(scr_e[:], eps5[0:1, 0:1], AF.Exp)

            # PE warm-up: lift the HAM clock gate before real matmuls arrive;
            # results are read once by a dummy copy so the buffer has a reader
            warm_ps = psmm.tile([128, 128], f32, tag="mm", name="warm")
            for wi in range(10):
                nc.tensor.matmul(warm_ps[:], ident_bf[:], ident_bf[:],
                                 start=True, stop=True)
            warm_sb = const.tile([1, 1], f32)
            nc.vector.tensor_copy(warm_sb[:], warm_ps[0:1, 0:1])


            x_all = pers.tile([128, NT, D], f32)
            x_r = x_d.rearrange("(n p) d -> p n d", p=128)
            nc.sync.dma_start(x_all[:, 0:1, :], x_r[:, 0:1, :])
            nc.sync.dma_start(x_all[:, 1:4, :], x_r[:, 1:4, :])
            # fp32 warmups reading x0: bridge the startup barrier so the PE
            # HAM window stays busy right up to the first real transposes
            warm2 = psmm.tile([128, D], f32, tag="mm", name="warm2")
            for wi in range(6):
                nc.tensor.matmul(warm2[:], ident[:], x_all[:, 0, :],
                                 start=True, stop=True)
            warm2_sb = const.tile([1, 1], f32)
            nc.vector.tensor_copy(warm2_sb[:], warm2[0:1, 0:1])

            def wload(dram, shape):
                t = const.tile(list(shape), bf, tag=dram.name)
                nc.sync.dma_start(t[:], dram[:])
                return t

            nc.sync.dma_start(aff[:], aff_d[:])
            wv = wload(wv_d, (128, 2, D))
            wqkv = wload(wqkv_d, (128, 2, 3 * D))
            for qg_ in range(1, 4):
                nc.sync.dma_start(x_all[:, qg_ * 4:(qg_ + 1) * 4, :],
                                  x_r[:, qg_ * 4:(qg_ + 1) * 4, :])
            wk = wload(wk_d, (128, 2, D))
            wq = wload(wq_d, (128, 2, D))
            lt_r = lt_d.rearrange("(n p) t -> p n t", p=128)
            lt_all = pers.tile([128, NT, SH], bf)
            for qg_ in range(4):
                nc.sync.dma_start(lt_all[:, qg_ * 4:(qg_ + 1) * 4, :],
                                  lt_r[:, qg_ * 4:(qg_ + 1) * 4, :])
            wf1 = wload(wf1_d, (128, 2, D))
            wf2 = wload(wf2_d, (128, 2, 2))
            wg1 = wload(wg1_d, (128, 4, D))
            wg2 = wload(wg2_d, (128, 2, D))
            wgo = wload(wgo_d, (128, 2, D))
            wff1 = wload(wff1_d, (128, 2, 4 * D))
            wff2 = wload(wff2_d, (128, 8, D))

            hT = pers.tile([128, 2, L], bf)    # h^T, full batch
            hKT = pers.tile([128, 2, L], bf)   # (h@Wk)^T, full batch
            hV = pers.tile([128, NT, D], bf)   # h@Wv@Wlo, token-major
            tap_tiles["hT"], tap_tiles["hKT"], tap_tiles["hV"] = hT, hKT, hV
            qT = pers.tile([128, 2, SH], bf)
            tap_tiles["qT"] = qT

            f1T = pers.tile([128, 2, SH], bf)
            qg_all = pers.tile([128, ST, D], f32)
            kv_ps = [psacc.tile([128, 257], f32, tag="acc", name=f"kv{g}")
                     for g in range(2)]
            agg_ps = [psacc.tile([128, 512], f32, tag="acc", name=f"agg{g}")
                      for g in range(2)]
            den_acc = pssml.tile([1, 512], f32, tag="accs", name="den")

            rstd_rest = pers.tile([128, 12], f32)
            nmr_rest = pers.tile([128, 12], f32)

            heron_n = [0]

            def heron_core(rstd_out, nmr_out, mean_ap, var_ap, iters, k):
                """rstd = 1/sqrt(var + 1e-5), nmr = -mean*rstd.  All-DVE
                Heron iterations (no ACT sqrt table)."""
                heron_n[0] += 1
                hid = heron_n[0]
                vh = stm.tile([128, k], f32, tag="her", name=f"vh_{hid}")
                nc.vector.tensor_scalar(vh[:], var_ap, 0.5, 5e-6,
                                        op0=ALU.mult, op1=ALU.add)
                s = stm.tile([128, k], f32, tag="her", name=f"s_{hid}")
                nc.vector.tensor_scalar_add(s[:], vh[:], 0.5)
                r = stm.tile([128, k], f32, tag="her", name=f"r_{hid}")
                q = stm.tile([128, k], f32, tag="her", name=f"q_{hid}")
                for _ in range(iters):
                    nc.vector.reciprocal(r[:], s[:])
                    nc.vector.tensor_mul(q[:], vh[:], r[:])
                    nc.vector.scalar_tensor_tensor(s[:], s[:], 0.5, q[:],
                                                   op0=ALU.mult, op1=ALU.add)
                nc.vector.reciprocal(rstd_out, s[:])
                nc.vector.scalar_tensor_tensor(nmr_out, mean_ap, -1.0,
                                               rstd_out, op0=ALU.mult,
                                               op1=ALU.mult)

            def heron_rstd(rstd_out, nmr_out, mean_c, msq_c, iters):
                k = mean_c.shape[-1]
                hid = heron_n[0] + 100
                m2 = stm.tile([128, k], f32, tag="her", name=f"m2_{hid}")
                nc.vector.tensor_mul(m2[:], mean_c[:], mean_c[:])
                df = stm.tile([128, k], f32, tag="her", name=f"df_{hid}")
                nc.vector.tensor_sub(df[:], msq_c[:], m2[:])
                heron_core(rstd_out, nmr_out, mean_c[:], df[:], iters, k)

            # producers for the software-pipelined accumulators
            kg_tiles = {}
            ut_tiles = {}

            def emit_kv(n):
                kg_l, vg_rhs = kg_tiles.pop(n)
                for g in range(2):
                    nc.tensor.matmul(kv_ps[g][:], kg_l[:, g * 128:(g + 1) * 128],
                                     vg_rhs[:], start=(n == 0), stop=(n == NT - 1))

            def emit_attn_acc(jc):
                ut = ut_tiles.pop(jc)
                for g in range(2):
                    nc.tensor.matmul(agg_ps[g][:], hV[:, jc, g * 128:(g + 1) * 128],
                                     ut[:], start=(jc == 0), stop=(jc == NT - 1))

            # ---------- fused pre-work + attention, per 4-tile group ----------
            for qgrp in range(4):
                tiles = range(qgrp * 4, qgrp * 4 + 4)
                if qgrp == 0:
                    mval = stm.tile([128, 4, 2], f32, tag="mval")
                    rstd4 = stm.tile([128, 4], f32, tag="rstd4")
                    nmr4 = stm.tile([128, 4], f32, tag="nmr4")
                    stats = stm.tile([128, 6], f32, tag="stats")
                    nc.vector.bn_stats(out=stats[:], in_=x_all[:, 0, :])
                    nc.vector.bn_aggr(out=mval[:, 0, :], in_=stats[:])
                    heron_core(rstd4[:, 0:1], nmr4[:, 0:1], mval[:, 0, 0:1],
                               mval[:, 0, 1:2], iters=3, k=1)
                else:
                    rstd4 = rstd_rest[:, (qgrp - 1) * 4:qgrp * 4]
                    nmr4 = nmr_rest[:, (qgrp - 1) * 4:qgrp * 4]
                for i, n in enumerate(tiles):
                    if qgrp == 0 and i == 1:
                        # stats for tiles 1-3 emitted after tile 0's chain so
                        # the first transpose isn't queued behind them
                        for i2 in range(1, 4):
                            stats = stm.tile([128, 6], f32, tag="stats")
                            nc.vector.bn_stats(out=stats[:], in_=x_all[:, i2, :])
                            nc.vector.bn_aggr(out=mval[:, i2, :], in_=stats[:])
                        heron_core(rstd4[:, 1:4], nmr4[:, 1:4], mval[:, 1:4, 0],
                                   mval[:, 1:4, 1], iters=3, k=3)
                    js = slice(n * 128, (n + 1) * 128)
                    # hn = (x - m) * rstd  (one fused DVE op, bf16 out)
                    hn = hnp.tile([128, D], f32, tag="hn")
                    nc.vector.tensor_scalar(hn[:], x_all[:, n, :],
                                            rstd4[:, i:i + 1], nmr4[:, i:i + 1],
                                            op0=ALU.mult, op1=ALU.add)
                    for c in range(2):
                        pt = psmm.tile([128, 128], f32, tag="mm")
                        nc.tensor.transpose(pt[:], hn[:, c * 128:(c + 1) * 128],
                                            ident[:])
                        # h = hn * g1 + b1 on the transposed copy-out (DVE)
                        nc.vector.tensor_scalar(hT[:, c, js], pt[:],
                                                aff[:, c, 0:1], aff[:, c, 1:2],
                                                op0=ALU.mult, op1=ALU.add)
                    # hV tile
                    pv = psmm.tile([128, D], f32, tag="mm")
                    for c in range(2):
                        nc.tensor.matmul(pv[:], hT[:, c, js], wv[:, c, :],
                                         start=(c == 0), stop=(c == 1))
                    nc.scalar.copy(hV[:, n, :], pv[:])
                    # kg/vg projection + elu(k)+1 = min(exp(k),1) + max(k,0)
                    pq = psmm.tile([128, 512], f32, tag="mm")
                    for c in range(2):
                        nc.tensor.matmul(pq[:], hT[:, c, js], wqkv[:, c, D:3 * D],
                                         start=(c == 0), stop=(c == 1))
                    te = stmq.tile([128, D], bf, tag="tmpq")
                    nc.scalar.activation(te[:], pq[:, 0:D], AF.Exp)
                    ta_ = stmq.tile([128, D], bf, tag="tmpq")
                    nc.vector.tensor_scalar_min(ta_[:], te[:], 1.0)
                    tr = stmq.tile([128, D], bf, tag="tmpq")
                    nc.vector.tensor_scalar_max(tr[:], pq[:, 0:D], 0.0)
                    kg_l = stm.tile([128, D], bf, tag="kg_l")
                    nc.vector.tensor_add(kg_l[:], ta_[:], tr[:])
                    vg_rhs = stm.tile([128, D + 1], bf, tag="vg_rhs")
                    nc.vector.tensor_copy(vg_rhs[:, 0:D], pq[:, D:2 * D])
                    nc.gpsimd.memset(vg_rhs[:, D:D + 1], 1.0)
                    kg_tiles[n] = (kg_l, vg_rhs)
                    if n >= 2:
                        emit_kv(n - 2)

                # hKT chunk for this group
                jsg = slice(qgrp * 512, (qgrp + 1) * 512)
                for g in range(2):
                    pk = psmm.tile([128, 512], f32, tag="mm")
                    for c in range(2):
                        nc.tensor.matmul(pk[:], wk[:, c, g * 128:(g + 1) * 128],
                                         hT[:, c, jsg], start=(c == 0), stop=(c == 1))
                    nc.scalar.copy(hKT[:, g, jsg], pk[:])
                # qT + linear-attn qg (needs hT tiles 0..3 only)
                if qgrp == 0:
                    for g in range(2):
                        pq2 = psmm.tile([128, 512], f32, tag="mm")
                        for c in range(2):
                            nc.tensor.matmul(pq2[:], wq[:, c, g * 128:(g + 1) * 128],
                                             hT[:, c, 0:SH], start=(c == 0), stop=(c == 1))
                        nc.vector.tensor_copy(qT[:, g, :], pq2[:])
                    for it in range(ST):
                        ts_ = slice(it * 128, (it + 1) * 128)
                        pq3 = psmm.tile([128, D], f32, tag="mm")
                        for c in range(2):
                            nc.tensor.matmul(pq3[:], hT[:, c, ts_], wqkv[:, c, 0:D],
                                             start=(c == 0), stop=(c == 1))
                        teb = stmq.tile([128, D], f32, tag="tmpq")
                        nc.scalar.activation(teb[:], pq3[:], AF.Exp)
                        tab_ = stmq.tile([128, D], f32, tag="tmpq")
                        nc.vector.tensor_scalar_min(tab_[:], teb[:], 1.0)
                        trb = stmq.tile([128, D], f32, tag="tmpq")
                        nc.vector.tensor_scalar_max(trb[:], pq3[:], 0.0)
                        nc.vector.tensor_add(qg_all[:, it, :], tab_[:], trb[:])
                    # batched LN stats for tiles 4..15 (DVE bn + Heron)
                    mv_r = stm.tile([128, 12, 2], f32, tag="mv_r")
                    for i2, n2 in enumerate(range(4, NT)):
                        stats2 = stm.tile([128, 6], f32, tag="stats")
                        nc.vector.bn_stats(out=stats2[:], in_=x_all[:, n2, :])
                        nc.vector.bn_aggr(out=mv_r[:, i2, :], in_=stats2[:])
                    heron_core(rstd_rest[:], nmr_rest[:], mv_r[:, :, 0],
                               mv_r[:, :, 1], iters=3, k=12)

                # attention chunks for this group (acc pipelined one behind)
                for jc in tiles:
                    js = slice(jc * 128, (jc + 1) * 128)
                    pl = psmm.tile([128, 512], f32, tag="mm")
                    nc.tensor.matmul(pl[:], hKT[:, 0, js], qT[:, 0, :],
                                     start=True, stop=False)
                    nc.tensor.matmul(pl[:], hKT[:, 1, js], qT[:, 1, :],
                                     start=False, stop=False)
                    nc.tensor.matmul(pl[:], ident_bf[:], lt_all[:, jc, :],
                                     start=False, stop=True)
                    ut = stm.tile([128, 512], bf, tag="ut")
                    nc.scalar.activation(ut[:], pl[:], AF.Exp)
                    nc.tensor.matmul(den_acc[:], ones_cb[:], ut[:],
                                     start=(jc == 0), stop=(jc == NT - 1))
                    ut_tiles[jc] = ut
                    if jc >= 2:
                        emit_attn_acc(jc - 2)
            emit_kv(NT - 2)
            emit_kv(NT - 1)
            emit_attn_acc(NT - 2)
            emit_attn_acc(NT - 1)

            # ---------- tail: reordered for cross-engine overlap -------------
            aggloT = pers.tile([128, 2, SH], bf)
            tap_tiles["aggloT"] = aggloT
            g1T = pers.tile([128, 2, SH], bf)
            tgate = pers.tile([128, 2, SH], bf)
            tap_tiles["tgate"] = tgate
            h_localT = pers.tile([128, 2, SH], f32)
            tap_tiles["h_localT"] = h_localT
            qgzT = pers.tile([128, 2, SH], bf)
            tap_tiles["qgzT"] = qgzT
            yT = pers.tile([128, 2, SH], bf)
            tap_tiles["yT"] = yT
            h_globalT = pers.tile([128, 2, SH], f32)
            tap_tiles["h_globalT"] = h_globalT
            xoT = pers.tile([128, 2, SH], f32)
            tap_tiles["xoT"] = xoT
            xo_bf = pers.tile([128, 2, SH], bf)
            xnT = pers.tile([128, 2, SH], bf)
            tap_tiles["xnT"] = xnT
            ff1T = pers.tile([128, 8, SH], bf)
            outT = pers.tile([128, 2, SH], f32)
            tap_tiles["outT"] = outT

            # kv block-diagonal + ksum extraction (ACT/DVE, first thing)
            kvb = pers.tile([128, 2, D], bf)
            tap_tiles["kvb"] = kvb
            nc.vector.memset(kvb[:], 0.0)
            for h in range(H):
                g, po = h // 4, (h * DH) % 128
                nc.scalar.copy(kvb[po:po + DH, g, h * DH:(h + 1) * DH],
                               kv_ps[g][po:po + DH, h * DH:(h + 1) * DH])
            ksum_col = pers.tile([128, 2], f32)
            for g in range(2):
                nc.vector.tensor_copy(ksum_col[:, g:g + 1], kv_ps[g][:, D:D + 1])
            ksum_row = pers.tile([1, D], bf)
            for g in range(2):
                pt = psmm.tile([128, 128], f32, tag="mm")
                nc.tensor.transpose(pt[0:1, 0:128], ksum_col[:, g:g + 1], ident[:])
                nc.vector.tensor_copy(ksum_row[0:1, g * 128:(g + 1) * 128],
                                      pt[0:1, 0:128])
            kb_ps = psmm.tile([128, D], f32, tag="mm")
            nc.tensor.matmul(kb_ps[:], ones_rb[:], ksum_row[:], start=True, stop=True)
            ksumb = pers.tile([128, D], bf)
            tap_tiles["ksumb"] = ksumb
            nc.vector.tensor_copy(ksumb[:], kb_ps[:])

            # gate first-level matmuls on h (independent of agglo) keep PE busy
            pgs = [psacc.tile([128, 512], f32, tag="acc", name=f"pg{g}")
                   for g in range(2)]
            for g in range(2):
                for c in range(2):
                    nc.tensor.matmul(pgs[g][:], wg1[:, c, g * 128:(g + 1) * 128],
                                     hT[:, c, 0:SH], start=(c == 0), stop=False)

            # denominator reciprocal + agglo
            den_sb2 = stm.tile([1, 512], f32, tag="den_sb2")
            nc.vector.tensor_copy(den_sb2[:], den_acc[:])
            den_rb = pers.tile([1, 512], bf)
            tap_tiles["den_rb"] = den_rb
            with nc.allow_low_precision("bf16 recip feeds bf16 broadcast"):
                nc.vector.reciprocal(den_rb[:], den_sb2[:])
            rbp = psmm.tile([128, 512], f32, tag="mm", name="rbp")
            nc.tensor.matmul(rbp[:], ones_rb[:], den_rb[:], start=True, stop=True)
            rbh = stmf.tile([128, 512], bf, tag="tmpf", name="rbh")
            nc.scalar.copy(rbh[:], rbp[:])
            hl_base = pers.tile([128, 2, SH], f32)
            for g in range(2):
                nc.vector.tensor_mul(aggloT[:, g, :], agg_ps[g][:], rbh[:])
                nc.vector.scalar_tensor_tensor(hl_base[:, g, :], aggloT[:, g, :],
                                               0.5, hT[:, g, 0:SH],
                                               op0=ALU.mult, op1=ALU.add)

            # f1 chain matmuls + first gelu (triggers the one gelu table load)
            for g in range(2):
                pf = psmm.tile([128, 512], f32, tag="mm")
                for c in range(2):
                    nc.tensor.matmul(pf[:], wf1[:, c, g * 128:(g + 1) * 128],
                                     hT[:, c, 0:SH], start=(c == 0), stop=(c == 1))
                nc.scalar.activation(f1T[:, g, :], pf[:], AF.Gelu)

            # gate second half (agglo) + g1 gelu
            for g in range(2):
                for c in range(2):
                    nc.tensor.matmul(pgs[g][:], wg1[:, 2 + c, g * 128:(g + 1) * 128],
                                     aggloT[:, c, :], start=False, stop=(c == 1))
                nc.scalar.activation(g1T[:, g, :], pgs[g][:], AF.Gelu)

            # linear attention z + qgz (DVE) overlapping the gate/f1 chains
            zden_a = stm.tile([128, ST, H], f32, tag="zden_a")
            for it in range(ST):
                prod = stmq.tile([128, D], f32, tag="tmpq")
                nc.vector.tensor_mul(prod[:], qg_all[:, it, :], ksumb[:])
                nc.vector.tensor_reduce(zden_a[:, it, :],
                                        prod[:].rearrange("p (h d) -> p h d", d=DH),
                                        axis=AX.X, op=ALU.add)
            nc.vector.tensor_scalar_add(zden_a[:], zden_a[:], 1e-6)
            zr_a = stm.tile([128, ST, H], f32, tag="zr_a")
            nc.vector.reciprocal(
                zr_a[:].rearrange("p a b -> p (a b)"),
                zden_a[:].rearrange("p a b -> p (a b)"))
            qgz_t = []
            for it in range(ST):
                qgz = pers.tile([128, D], f32, name=f"qgz{it}")
                nc.vector.tensor_tensor(
                    out=qgz[:].rearrange("p (h d) -> p h d", d=DH),
                    in0=qg_all[:, it, :].rearrange("p (h d) -> p h d", d=DH),
                    in1=zr_a[:, it, :].to_broadcast([128, H, DH]), op=ALU.mult)
                qgz_t.append(qgz)
            # wf fuse-gate weights
            wf_ps = pssml.tile([2, 512], f32, tag="accs", name="wfps")
            wf_sb = stm.tile([2, 512], bf, tag="wf_sb")
            for c in range(2):
                nc.tensor.matmul(wf_ps[:], wf2[:, c, :], f1T[:, c, :],
                                 start=(c == 0), stop=(c == 1))
            nc.scalar.copy(wf_sb[:], wf_ps[:])
            d01_ps = psmm.tile([1, 512], f32, tag="mm", name="d01")
            nc.tensor.matmul(d01_ps[:], pm[:], wf_sb[:], start=True, stop=True)
            th_wf = stm.tile([1, 512], bf, tag="th_wf")
            nc.scalar.activation(th_wf[:], d01_ps[:], AF.Tanh, scale=0.5)
            wf0 = pers.tile([1, 512], bf)
            tap_tiles["wf0"] = wf0
            wf1s = pers.tile([1, 512], bf)
            nc.vector.tensor_scalar(wf0[:], th_wf[:], 0.5, 0.5,
                                    op0=ALU.mult, op1=ALU.add)
            nc.vector.tensor_scalar(wf1s[:], th_wf[:], -0.5, 0.5,
                                    op0=ALU.mult, op1=ALU.add)

            # gate second level + tanh; y and h_global in parallel
            for g in range(2):
                pg2 = psmm.tile([128, 512], f32, tag="mm")
                for c in range(2):
                    nc.tensor.matmul(pg2[:], wg2[:, c, g * 128:(g + 1) * 128],
                                     g1T[:, c, :], start=(c == 0), stop=(c == 1))
                nc.scalar.activation(tgate[:, g, :], pg2[:], AF.Tanh, scale=0.5)
            for it in range(ST):
                ts_ = slice(it * 128, (it + 1) * 128)
                for c in range(2):
                    ptq = psmm.tile([128, 128], f32, tag="mm")
                    nc.tensor.transpose(ptq[:], qgz_t[it][:, c * 128:(c + 1) * 128],
                                        ident[:])
                    nc.vector.tensor_copy(qgzT[:, c, ts_], ptq[:])
            for g in range(2):
                py = psmm.tile([128, 512], f32, tag="mm")
                for c in range(2):
                    nc.tensor.matmul(py[:], kvb[:, c, g * 128:(g + 1) * 128],
                                     qgzT[:, c, :], start=(c == 0), stop=(c == 1))
                nc.vector.tensor_copy(yT[:, g, :], py[:])
            for g in range(2):
                pgo = psmm.tile([128, 512], f32, tag="mm")
                for c in range(2):
                    nc.tensor.matmul(pgo[:], wgo[:, c, g * 128:(g + 1) * 128],
                                     yT[:, c, :], start=(c == 0), stop=(c == 1))
                nc.vector.tensor_add(h_globalT[:, g, :], hT[:, g, 0:SH], pgo[:])
            # h_local = (h + 0.5*agglo) + 0.5*(agglo*tanh)
            for g in range(2):
                w = stmf.tile([128, 512], bf, tag="tmpf")
                nc.vector.tensor_mul(w[:], tgate[:, g, :], aggloT[:, g, :])
                nc.vector.scalar_tensor_tensor(h_localT[:, g, :], w[:], 0.5,
                                               hl_base[:, g, :],
                                               op0=ALU.mult, op1=ALU.add)

            # xo = wf0*h_local + wf1*h_global
            b0p = psmm.tile([128, 512], f32, tag="mm", name="b0p")
            nc.tensor.matmul(b0p[:], ones_rb[:], wf0[:], start=True, stop=True)
            b1p = psmm.tile([128, 512], f32, tag="mm", name="b1p")
            nc.tensor.matmul(b1p[:], ones_rb[:], wf1s[:], start=True, stop=True)
            for g in range(2):
                ta = stmf.tile([128, 512], f32, tag="tmpf")
                nc.vector.tensor_mul(ta[:], h_localT[:, g, :], b0p[:])
                tb = stmf.tile([128, 512], f32, tag="tmpf")
                nc.vector.tensor_mul(tb[:], h_globalT[:, g, :], b1p[:])
                nc.vector.tensor_add(xoT[:, g, :], ta[:], tb[:])
                nc.vector.tensor_copy(xo_bf[:, g, :], xoT[:, g, :])

            # ---------- LN2: stats via matmul, rsqrt via column Heron ---------
            sum_ps = pssml.tile([1, 512], f32, tag="accs", name="sumps")
            for c in range(2):
                nc.tensor.matmul(sum_ps[:], ones_cb[:], xo_bf[:, c, :],
                                 start=(c == 0), stop=(c == 1))
            ssq_ps = psmm.tile([1, 512], f32, tag="mm", name="ssqps")
            for c in range(2):
                xsq = stmf.tile([128, 512], bf, tag="xsq")
                nc.scalar.activation(xsq[:], xoT[:, c, :], AF.Square)
                nc.tensor.matmul(ssq_ps[:], ones_cb[:], xsq[:],
                                 start=(c == 0), stop=(c == 1))
            sum_row = stm.tile([1, 512], f32, tag="sum_row")
            nc.vector.tensor_copy(sum_row[:], sum_ps[:])
            ssq_row = stm.tile([1, 512], f32, tag="ssq_row")
            nc.vector.tensor_copy(ssq_row[:], ssq_ps[:])
            sq_col = stm.tile([128, 4, 2], f32, tag="sq_col")
            for kk in range(4):
                pts = psmm.tile([128, 128], f32, tag="mm")
                nc.tensor.transpose(pts[0:128, 0:1],
                                    sum_row[0:1, kk * 128:(kk + 1) * 128],
                                    ident[0:1, 0:1])
                nc.vector.tensor_copy(sq_col[:, kk, 0:1], pts[0:128, 0:1])
                ptq2 = psmm.tile([128, 128], f32, tag="mm")
                nc.tensor.transpose(ptq2[0:128, 0:1],
                                    ssq_row[0:1, kk * 128:(kk + 1) * 128],
                                    ident[0:1, 0:1])
                nc.vector.tensor_copy(sq_col[:, kk, 1:2], ptq2[0:128, 0:1])
            mean_c = stm.tile([128, 4], f32, tag="mean_c")
            nc.vector.tensor_scalar_mul(mean_c[:], sq_col[:, :, 0], 1.0 / D)
            msq_c = stm.tile([128, 4], f32, tag="msq_c")
            nc.vector.tensor_scalar_mul(msq_c[:], sq_col[:, :, 1], 1.0 / D)
            rstd_c = stm.tile([128, 4], f32, tag="rstd_c")
            nmr_c = stm.tile([128, 4], f32, tag="nmr_c")
            heron_rstd(rstd_c[:], nmr_c[:], mean_c, msq_c, iters=5)
            nmr_row = stm.tile([1, 512], bf, tag="nmr_row")
            rstd_row = stm.tile([1, 512], bf, tag="rstd_row")
            for kk in range(4):
                ptb0 = psmm.tile([128, 128], f32, tag="mm")
                nc.tensor.transpose(ptb0[0:1, 0:128], nmr_c[:, kk:kk + 1],
                                    ident[:])
                nc.vector.tensor_copy(nmr_row[:, kk * 128:(kk + 1) * 128],
                                      ptb0[0:1, 0:128])
                ptb1 = psmm.tile([128, 128], f32, tag="mm")
                nc.tensor.transpose(ptb1[0:1, 0:128], rstd_c[:, kk:kk + 1],
                                    ident[:])
                nc.vector.tensor_copy(rstd_row[:, kk * 128:(kk + 1) * 128],
                                      ptb1[0:1, 0:128])
            nmr_b = psmm.tile([128, 512], f32, tag="mm", name="nmrb")
            nc.tensor.matmul(nmr_b[:], ones_rb[:], nmr_row[:],
                             start=True, stop=True)
            rb2_b = psmm.tile([128, 512], f32, tag="mm", name="rb2b")
            nc.tensor.matmul(rb2_b[:], ones_rb[:], rstd_row[:],
                             start=True, stop=True)
            for g in range(2):
                t1 = stmf.tile([128, 512], f32, tag="tmpf")
                nc.vector.tensor_mul(t1[:], xoT[:, g, :], rb2_b[:])
                # g2 is folded into Wff1 on the host; write bf16 directly
                nc.vector.tensor_add(xnT[:, g, :], t1[:], nmr_b[:])

            # FFN + residual + output transposes/stores
            pf2s = [psacc.tile([128, 512], f32, tag="acc", name=f"pf2_{g}")
                    for g in range(2)]
            for g8 in range(8):
                pff = psmm.tile([128, 512], f32, tag="mm")
                for c in range(2):
                    nc.tensor.matmul(pff[:], wff1[:, c, g8 * 128:(g8 + 1) * 128],
                                     xnT[:, c, :], start=(c == 0), stop=(c == 1))
                nc.scalar.activation(ff1T[:, g8, :], pff[:], AF.Gelu)
                if g8 >= 1:
                    for g in range(2):
                        nc.tensor.matmul(pf2s[g][:],
                                         wff2[:, g8 - 1, g * 128:(g + 1) * 128],
                                         ff1T[:, g8 - 1, :],
                                         start=(g8 == 1), stop=False)
            for g in range(2):
                nc.tensor.matmul(pf2s[g][:], wff2[:, 7, g * 128:(g + 1) * 128],
                                 ff1T[:, 7, :], start=False, stop=True)
                nc.vector.tensor_add(outT[:, g, :], xoT[:, g, :], pf2s[g][:])
            for it in range(ST):
                ts_ = slice(it * 128, (it + 1) * 128)
                ot = stmq.tile([128, D], f32, tag="tmpq")
                for c in range(2):
                    pto = psmm.tile([128, 128], f32, tag="mm")
                    nc.tensor.transpose(pto[:], outT[:, c, ts_], ident[:])
                    nc.vector.tensor_copy(ot[:, c * 128:(c + 1) * 128], pto[:])
                nc.sync.dma_start(out_d[ts_, :], ot[:])

            for name in taps:
                t = tap_tiles[name]
                td = nc.dram_tensor(f"tap_{name}", list(t.shape),
                                    t.dtype, kind="ExternalOutput")
                nc.sync.dma_start(td[:], t[:])

    nc.compile()
    return nc


def _host_prep(inputs):
    """Host-side preprocessing shared by all cores + per-core arrays."""
    x = np.asarray(inputs["x"], np.float32)
    mask = np.asarray(inputs["mask"])
    nbr_idx = np.asarray(inputs["nbr_idx"]).astype(np.int64)
    nbr_mask = np.asarray(inputs["nbr_mask"])
    rel_pos = np.asarray(inputs["rel_pos"]).astype(np.int64)

    if not (np.all(mask == 1)):
        raise NotImplementedError("kernel assumes mask == ones (spec fill)")

    # edge-bias table over the 65 possible rel values
    Erel = np.asarray(inputs["Erel"], np.float32)
    We1 = np.asarray(inputs["We1"], np.float32)
    be1 = np.asarray(inputs["be1"], np.float32)
    We2 = np.asarray(inputs["We2"], np.float32)
    be2 = np.asarray(inputs["be2"], np.float32)
    tab = (_gelu_np(Erel @ We1 + be1) @ We2 + be2)[:, 0]  # [65]

    rel = np.clip(rel_pos, -CLIP, CLIP) + CLIP
    ev = np.exp(tab[rel]) * (nbr_mask != 0)  # [B, L, K]

    # dense E^T per batch: ET[b][j, t] = sum_k ev[b,t,k] * [idx==j]
    ET = np.zeros((B, L, L), np.float32)
    for b in range(B):
        t_idx = np.repeat(np.arange(L), K)
        np.add.at(ET[b], (nbr_idx[b].ravel(), t_idx), ev[b].ravel())
    # log-domain (folded into the score PSUM in-kernel): -1e30 where empty
    LT = np.where(ET > 0, np.log(np.maximum(ET, 1e-30)), -1e30).astype(np.float32)

    aff = np.zeros((128, 2, 4), np.float32)
    for name, i in (("g1", 0), ("b1", 1), ("g2", 2), ("b2", 3)):
        v = np.asarray(inputs[name], np.float32)
        aff[:, :, i] = v.reshape(2, 128).T

    shared = {
        "aff": aff,
        "wq": _w_tiles(np.asarray(inputs["Wq"], np.float32) / 16.0, 2),
        "wk": _w_tiles(np.asarray(inputs["Wk"], np.float32), 2),
        "wv": _w_tiles(np.asarray(inputs["Wv"], np.float32)
                       @ np.asarray(inputs["Wlo"], np.float32), 2),
        "wg1": _w_tiles(np.asarray(inputs["Wg1"], np.float32), 4),
        "wg2": _w_tiles(np.asarray(inputs["Wg2"], np.float32), 2),
        "wqkv": _w_tiles(np.asarray(inputs["Wqkv"], np.float32), 2),
        "wgo": _w_tiles(np.asarray(inputs["Wgo"], np.float32), 2),
        "wf1": _w_tiles(np.asarray(inputs["Wf1"], np.float32), 2),
        "wf2": _w_tiles(np.asarray(inputs["Wf2"], np.float32), 2),
        "wff1": _w_tiles(np.asarray(inputs["g2"], np.float32)[:, None]
                         * np.asarray(inputs["Wff1"], np.float32), 2),
        "wff2": _w_tiles(np.asarray(inputs["Wff2"], np.float32), 8),
        "pm": np.array([[1.0], [-1.0]], BF16),
    }
    for k in ("blo", "bg1", "bg2", "bf1", "bf2", "bff1", "bff2", "b2"):
        if not np.allclose(np.asarray(inputs[k]), 0.0):
            raise NotImplementedError(f"kernel assumes bias {k} == 0 (spec fill)")

    per_core = []
    for c in range(NCORES):
        b, s = c // SPB, c % SPB
        s0 = s * SH
        xp = np.roll(x[b], -s0, axis=0)
        ltp = np.roll(LT[b][:, s0:s0 + SH], -s0, axis=0).astype(BF16)
        per_core.append({"x": np.ascontiguousarray(xp),
                         "lt": np.ascontiguousarray(ltp)})
    return shared, per_core


def kernel(**inputs) -> np.ndarray:
    import concourse.bass_utils as bu

    if "nc" not in _CACHE:
        _CACHE["nc"] = _build()
    nc = _CACHE["nc"]

    shared, per_core = _host_prep(inputs)
    in_maps = [{**shared, **pc} for pc in per_core]
    res = bu.run_bass_kernel_spmd(nc, in_maps, core_ids=list(range(NCORES)))
    out = np.zeros((B, L, D), np.float32)
    for c in range(NCORES):
        b, s = c // SPB, c % SPB
        out[b, s * SH:(s + 1) * SH] = res.results[c]["out"]
    return out


# revision 26
# speedup vs baseline: 1.2039x; 1.0362x over previous
"""Trainium2 Bass kernel for nn_Druggability_DistillModel (gnn_message_passing).

Strategy (8 NeuronCores, data-parallel over B x 4-way sequence shards):
  - core c handles batch b=c//4, tokens [s*512, (s+1)*512) with s=c%4.
  - The edge-bias MLP depends only on rel_pos (65 values) -> host collapses
    it to a table and builds LT[j, t] = log(sum_dup exp(edge)) over neighbors
    (−1e30 where none), so softmax_k(q.k/16 + edge) * v becomes
    exp(q.hK^T + LT) @ hV / rowsum — dense PE work, no gather.
  - LT is folded into the score PSUM via an identity matmul, so the sweep is
    matmul→matmul→matmul→Exp with no elementwise hop in between.
  - Denominators accumulate as rows of one [16,512] PSUM tile (one matmul per
    j-tile) and reduce with a single ones^T matmul at the end.
  - ACT table discipline: the scalar engine only ever loads the exp set (up
    front, via a dummy op that overlaps the first DMAs) and the gelu set (for
    the tail: gelu + tanh-as-sigmoid + square).  All rsqrt work (both
    layernorms) runs on the DVE as Heron iterations seeded from (1+v)/2.
  - PE warm-up: a burst of identity matmuls at t~0 lifts the HAM clock gate
    to full rate before the real prework arrives.
"""
import sys

sys.path.insert(0, "/opt/trn_rl_repo")

import math
import numpy as np
import ml_dtypes

B, L, D, H, DH, K, DE, CLIP = 2, 2048, 256, 8, 32, 36, 64, 32
NCORES, SPB, SH = 8, 4, 512  # cores, shards/batch, tokens/shard
NT = L // 128                # 16 token tiles per batch
ST = SH // 128               # 4 tiles per shard
BF16 = ml_dtypes.bfloat16

_CACHE: dict = {}


def _gelu_np(x):
    try:
        from scipy.special import erf
        e = erf(x / np.sqrt(2.0))
    except Exception:
        import math as _m
        e = np.vectorize(_m.erf)(x / np.sqrt(2.0))
    return x * 0.5 * (1.0 + e)


def _w_tiles(w, cin_chunks):
    """[din, dout] -> [128, cin_chunks, dout] with din = c*128+p."""
    din, dout = w.shape
    assert din == cin_chunks * 128
    return np.ascontiguousarray(
        w.reshape(cin_chunks, 128, dout).transpose(1, 0, 2)
    ).astype(BF16)


def _build(taps=()):
    import concourse.bass as bass
    import concourse.tile as tile
    from concourse import bacc, mybir
    from concourse.masks import make_identity

    f32, bf = mybir.dt.float32, mybir.dt.bfloat16
    AF = mybir.ActivationFunctionType
    ALU = mybir.AluOpType
    AX = mybir.AxisListType

    nc = bacc.Bacc("TRN2", target_bir_lowering=False, debug=False)

    x_d = nc.dram_tensor("x", [L, D], f32, kind="ExternalInput")
    lt_d = nc.dram_tensor("lt", [L, SH], bf, kind="ExternalInput")
    aff_d = nc.dram_tensor("aff", [128, 2, 4], f32, kind="ExternalInput")
    wq_d = nc.dram_tensor("wq", [128, 2, D], bf, kind="ExternalInput")
    wk_d = nc.dram_tensor("wk", [128, 2, D], bf, kind="ExternalInput")
    wv_d = nc.dram_tensor("wv", [128, 2, D], bf, kind="ExternalInput")
    wg1_d = nc.dram_tensor("wg1", [128, 4, D], bf, kind="ExternalInput")
    wg2_d = nc.dram_tensor("wg2", [128, 2, D], bf, kind="ExternalInput")
    wqkv_d = nc.dram_tensor("wqkv", [128, 2, 3 * D], bf, kind="ExternalInput")
    wgo_d = nc.dram_tensor("wgo", [128, 2, D], bf, kind="ExternalInput")
    wf1_d = nc.dram_tensor("wf1", [128, 2, D], bf, kind="ExternalInput")
    wf2_d = nc.dram_tensor("wf2", [128, 2, 2], bf, kind="ExternalInput")
    wff1_d = nc.dram_tensor("wff1", [128, 2, 4 * D], bf, kind="ExternalInput")
    wff2_d = nc.dram_tensor("wff2", [128, 8, D], bf, kind="ExternalInput")
    pm_d = nc.dram_tensor("pm", [2, 1], bf, kind="ExternalInput")
    out_d = nc.dram_tensor("out", [SH, D], f32, kind="ExternalOutput")
    tap_tiles = {}

    with tile.TileContext(nc) as tc:
        with (
            tc.tile_pool(name="const", bufs=1) as const,
            tc.tile_pool(name="persist", bufs=1) as pers,
            tc.tile_pool(name="stream", bufs=4) as stm,
            tc.tile_pool(name="stmf", bufs=3) as stmf,
            tc.tile_pool(name="stmq", bufs=8) as stmq,
            tc.tile_pool(name="hnp", bufs=3) as hnp,
            tc.tile_pool(name="psmm", bufs=3, space="PSUM") as psmm,
            tc.tile_pool(name="psacc", bufs=4, space="PSUM") as psacc,
            tc.tile_pool(name="pssml", bufs=1, space="PSUM") as pssml,
        ):
            ident = const.tile([128, 128], f32)
            make_identity(nc, ident[:])
            ident_bf = const.tile([128, 128], bf)
            make_identity(nc, ident_bf[:])
            ones_cb = const.tile([128, 1], bf)
            nc.vector.memset(ones_cb[:], 1.0)
            ones_rb = const.tile([1, 128], bf)
            nc.vector.memset(ones_rb[:], 1.0)
            pm = const.tile([2, 1], bf)
            nc.sync.dma_start(pm[:], pm_d[:])
            eps5 = const.tile([128, 1], f32)
            nc.vector.memset(eps5[:], 1e-5)
            ones_f1 = const.tile([1, 1], f32)
            nc.vector.memset(ones_f1[:], 1.0)
            aff = const.tile([128, 2, 4], f32)

            # preload the EXP activation table while DMAs stream in
            scr_e = const.tile([1, 1], f32)
            nc.scalar.activation(scr_e[:], eps5[0:1, 0:1], AF.Exp)

            # PE warm-up: lift the HAM clock gate before real matmuls arrive;
            # results are read once by a dummy copy so the buffer has a reader
            warm_ps = psmm.tile([128, 128], f32, tag="mm", name="warm")
            for wi in range(10):
                nc.tensor.matmul(warm_ps[:], ident_bf[:], ident_bf[:],
                                 start=True, stop=True)
            warm_sb = const.tile([1, 1], f32)
            nc.vector.tensor_copy(warm_sb[:], warm_ps[0:1, 0:1])


            x_all = pers.tile([128, NT, D], f32)
            x_r = x_d.rearrange("(n p) d -> p n d", p=128)
            nc.sync.dma_start(x_all[:, 0:1, :], x_r[:, 0:1, :])
            nc.sync.dma_start(x_all[:, 1:4, :], x_r[:, 1:4, :])
            # fp32 warmups reading x0: bridge the startup barrier so the PE
            # HAM window stays busy right up to the first real transposes
            warm2 = psmm.tile([128, D], f32, tag="mm", name="warm2")
            for wi in range(6):
                nc.tensor.matmul(warm2[:], ident[:], x_all[:, 0, :],
                                 start=True, stop=True)
            warm2_sb = const.tile([1, 1], f32)
            nc.vector.tensor_copy(warm2_sb[:], warm2[0:1, 0:1])

            def wload(dram, shape):
                t = const.tile(list(shape), bf, tag=dram.name)
                nc.sync.dma_start(t[:], dram[:])
                return t

            nc.sync.dma_start(aff[:], aff_d[:])
            wv = wload(wv_d, (128, 2, D))
            wqkv = wload(wqkv_d, (128, 2, 3 * D))
            for qg_ in range(1, 4):
                nc.sync.dma_start(x_all[:, qg_ * 4:(qg_ + 1) * 4, :],
                                  x_r[:, qg_ * 4:(qg_ + 1) * 4, :])
            wk = wload(wk_d, (128, 2, D))
            wq = wload(wq_d, (128, 2, D))
            lt_r = lt_d.rearrange("(n p) t -> p n t", p=128)
            lt_all = pers.tile([128, NT, SH], bf)
            for qg_ in range(4):
                nc.sync.dma_start(lt_all[:, qg_ * 4:(qg_ + 1) * 4, :],
                                  lt_r[:, qg_ * 4:(qg_ + 1) * 4, :])
            wf1 = wload(wf1_d, (128, 2, D))
            wf2 = wload(wf2_d, (128, 2, 2))
            wg1 = wload(wg1_d, (128, 4, D))
            wg2 = wload(wg2_d, (128, 2, D))
            wgo = wload(wgo_d, (128, 2, D))
            wff1 = wload(wff1_d, (128, 2, 4 * D))
            wff2 = wload(wff2_d, (128, 8, D))

            hT = pers.tile([128, 2, L], bf)    # h^T, full batch
            hV = pers.tile([128, NT, D], bf)   # h@Wv@Wlo, token-major
            tap_tiles["hT"], tap_tiles["hV"] = hT, hV
            qT = pers.tile([128, 2, SH], bf)
            tap_tiles["qT"] = qT
            kq = pers.tile([128, 2, SH], bf)   # Wk @ q^T  (replaces hKT)
            tap_tiles["kq"] = kq

            f1T = pers.tile([128, 2, SH], bf)
            qg_all = pers.tile([128, ST, D], f32)
            kv_ps = [psacc.tile([128, 257], f32, tag="acc", name=f"kv{g}")
                     for g in range(2)]
            agg_ps = [psacc.tile([128, 512], f32, tag="acc", name=f"agg{g}")
                      for g in range(2)]
            den_acc = pssml.tile([1, 512], f32, tag="accs", name="den")

            rstd_rest = pers.tile([128, 12], f32)
            nmr_rest = pers.tile([128, 12], f32)

            heron_n = [0]

            def heron_core(rstd_out, nmr_out, mean_ap, var_ap, iters, k):
                """rstd = 1/sqrt(var + 1e-5), nmr = -mean*rstd.  All-DVE
                Heron iterations (no ACT sqrt table)."""
                heron_n[0] += 1
                hid = heron_n[0]
                vh = stm.tile([128, k], f32, tag="her", name=f"vh_{hid}")
                nc.vector.tensor_scalar(vh[:], var_ap, 0.5, 5e-6,
                                        op0=ALU.mult, op1=ALU.add)
                s = stm.tile([128, k], f32, tag="her", name=f"s_{hid}")
                nc.vector.tensor_scalar_add(s[:], vh[:], 0.5)
                r = stm.tile([128, k], f32, tag="her", name=f"r_{hid}")
                q = stm.tile([128, k], f32, tag="her", name=f"q_{hid}")
                for _ in range(iters):
                    nc.vector.reciprocal(r[:], s[:])
                    nc.vector.tensor_mul(q[:], vh[:], r[:])
                    nc.vector.scalar_tensor_tensor(s[:], s[:], 0.5, q[:],
                                                   op0=ALU.mult, op1=ALU.add)
                nc.vector.reciprocal(rstd_out, s[:])
                nc.vector.scalar_tensor_tensor(nmr_out, mean_ap, -1.0,
                                               rstd_out, op0=ALU.mult,
                                               op1=ALU.mult)

            def heron_rstd(rstd_out, nmr_out, mean_c, msq_c, iters):
                k = mean_c.shape[-1]
                hid = heron_n[0] + 100
                m2 = stm.tile([128, k], f32, tag="her", name=f"m2_{hid}")
                nc.vector.tensor_mul(m2[:], mean_c[:], mean_c[:])
                df = stm.tile([128, k], f32, tag="her", name=f"df_{hid}")
                nc.vector.tensor_sub(df[:], msq_c[:], m2[:])
                heron_core(rstd_out, nmr_out, mean_c[:], df[:], iters, k)

            # producers for the software-pipelined accumulators
            kg_tiles = {}
            ut_tiles = {}

            def emit_kv(n):
                kg_l, vg_rhs = kg_tiles.pop(n)
                for g in range(2):
                    nc.tensor.matmul(kv_ps[g][:], kg_l[:, g * 128:(g + 1) * 128],
                                     vg_rhs[:], start=(n == 0), stop=(n == NT - 1))

            def emit_attn_acc(jc):
                ut = ut_tiles.pop(jc)
                for g in range(2):
                    nc.tensor.matmul(agg_ps[g][:], hV[:, jc, g * 128:(g + 1) * 128],
                                     ut[:], start=(jc == 0), stop=(jc == NT - 1))

            # ---------- fused pre-work + attention, per 4-tile group ----------
            for qgrp in range(4):
                tiles = range(qgrp * 4, qgrp * 4 + 4)
                if qgrp == 0:
                    mval = stm.tile([128, 4, 2], f32, tag="mval")
                    rstd4 = stm.tile([128, 4], f32, tag="rstd4")
                    nmr4 = stm.tile([128, 4], f32, tag="nmr4")
                    stats = stm.tile([128, 6], f32, tag="stats")
                    nc.vector.bn_stats(out=stats[:], in_=x_all[:, 0, :])
                    nc.vector.bn_aggr(out=mval[:, 0, :], in_=stats[:])
                    heron_core(rstd4[:, 0:1], nmr4[:, 0:1], mval[:, 0, 0:1],
                               mval[:, 0, 1:2], iters=3, k=1)
                else:
                    rstd4 = rstd_rest[:, (qgrp - 1) * 4:qgrp * 4]
                    nmr4 = nmr_rest[:, (qgrp - 1) * 4:qgrp * 4]
                for i, n in enumerate(tiles):
                    if qgrp == 0 and i == 1:
                        # stats for tiles 1-3 emitted after tile 0's chain so
                        # the first transpose isn't queued behind them
                        for i2 in range(1, 4):
                            stats = stm.tile([128, 6], f32, tag="stats")
                            nc.vector.bn_stats(out=stats[:], in_=x_all[:, i2, :])
                            nc.vector.bn_aggr(out=mval[:, i2, :], in_=stats[:])
                        heron_core(rstd4[:, 1:4], nmr4[:, 1:4], mval[:, 1:4, 0],
                                   mval[:, 1:4, 1], iters=3, k=3)
                    js = slice(n * 128, (n + 1) * 128)
                    # hn = (x - m) * rstd  (one fused DVE op, bf16 out)
                    hn = hnp.tile([128, D], f32, tag="hn")
                    nc.vector.tensor_scalar(hn[:], x_all[:, n, :],
                                            rstd4[:, i:i + 1], nmr4[:, i:i + 1],
                                            op0=ALU.mult, op1=ALU.add)
                    for c in range(2):
                        pt = psmm.tile([128, 128], f32, tag="mm")
                        nc.tensor.transpose(pt[:], hn[:, c * 128:(c + 1) * 128],
                                            ident[:])
                        # h = hn * g1 + b1 on the transposed copy-out (DVE)
                        nc.vector.tensor_scalar(hT[:, c, js], pt[:],
                                                aff[:, c, 0:1], aff[:, c, 1:2],
                                                op0=ALU.mult, op1=ALU.add)
                    # hV tile
                    pv = psmm.tile([128, D], f32, tag="mm")
                    for c in range(2):
                        nc.tensor.matmul(pv[:], hT[:, c, js], wv[:, c, :],
                                         start=(c == 0), stop=(c == 1))
                    nc.scalar.copy(hV[:, n, :], pv[:])
                    # kg/vg projection + elu(k)+1 = min(exp(k),1) + max(k,0)
                    pq = psmm.tile([128, 512], f32, tag="mm")
                    for c in range(2):
                        nc.tensor.matmul(pq[:], hT[:, c, js], wqkv[:, c, D:3 * D],
                                         start=(c == 0), stop=(c == 1))
                    te = stmq.tile([128, D], bf, tag="tmpq")
                    nc.scalar.activation(te[:], pq[:, 0:D], AF.Exp)
                    ta_ = stmq.tile([128, D], bf, tag="tmpq")
                    nc.vector.tensor_scalar_min(ta_[:], te[:], 1.0)
                    tr = stmq.tile([128, D], bf, tag="tmpq")
                    nc.vector.tensor_scalar_max(tr[:], pq[:, 0:D], 0.0)
                    kg_l = stm.tile([128, D], bf, tag="kg_l")
                    nc.vector.tensor_add(kg_l[:], ta_[:], tr[:])
                    vg_rhs = stm.tile([128, D + 1], bf, tag="vg_rhs")
                    nc.vector.tensor_copy(vg_rhs[:, 0:D], pq[:, D:2 * D])
                    nc.gpsimd.memset(vg_rhs[:, D:D + 1], 1.0)
                    kg_tiles[n] = (kg_l, vg_rhs)
                    if n >= 2:
                        emit_kv(n - 2)

                # qT + linear-attn qg (needs hT tiles 0..3 only)
                if qgrp == 0:
                    for g in range(2):
                        pq2 = psmm.tile([128, 512], f32, tag="mm")
                        for c in range(2):
                            nc.tensor.matmul(pq2[:], wq[:, c, g * 128:(g + 1) * 128],
                                             hT[:, c, 0:SH], start=(c == 0), stop=(c == 1))
                        nc.vector.tensor_copy(qT[:, g, :], pq2[:])
                    for g in range(2):
                        pkq = psmm.tile([128, 512], f32, tag="mm")
                        for c in range(2):
                            nc.tensor.matmul(pkq[:],
                                             wk[:, c, g * 128:(g + 1) * 128],
                                             qT[:, c, :], start=(c == 0),
                                             stop=(c == 1))
                        nc.scalar.copy(kq[:, g, :], pkq[:])
                    for it in range(ST):
                        ts_ = slice(it * 128, (it + 1) * 128)
                        pq3 = psmm.tile([128, D], f32, tag="mm")
                        for c in range(2):
                            nc.tensor.matmul(pq3[:], hT[:, c, ts_], wqkv[:, c, 0:D],
                                             start=(c == 0), stop=(c == 1))
                        teb = stmq.tile([128, D], f32, tag="tmpq")
                        nc.scalar.activation(teb[:], pq3[:], AF.Exp)
                        tab_ = stmq.tile([128, D], f32, tag="tmpq")
                        nc.vector.tensor_scalar_min(tab_[:], teb[:], 1.0)
                        trb = stmq.tile([128, D], f32, tag="tmpq")
                        nc.vector.tensor_scalar_max(trb[:], pq3[:], 0.0)
                        nc.vector.tensor_add(qg_all[:, it, :], tab_[:], trb[:])
                    # batched LN stats for tiles 4..15 (DVE bn + Heron)
                    mv_r = stm.tile([128, 12, 2], f32, tag="mv_r")
                    for i2, n2 in enumerate(range(4, NT)):
                        stats2 = stm.tile([128, 6], f32, tag="stats")
                        nc.vector.bn_stats(out=stats2[:], in_=x_all[:, n2, :])
                        nc.vector.bn_aggr(out=mv_r[:, i2, :], in_=stats2[:])
                    heron_core(rstd_rest[:], nmr_rest[:], mv_r[:, :, 0],
                               mv_r[:, :, 1], iters=3, k=12)

                # attention chunks for this group (acc pipelined one behind)
                for jc in tiles:
                    js = slice(jc * 128, (jc + 1) * 128)
                    pl = psmm.tile([128, 512], f32, tag="mm")
                    nc.tensor.matmul(pl[:], hT[:, 0, js], kq[:, 0, :],
                                     start=True, stop=False)
                    nc.tensor.matmul(pl[:], hT[:, 1, js], kq[:, 1, :],
                                     start=False, stop=False)
                    nc.tensor.matmul(pl[:], ident_bf[:], lt_all[:, jc, :],
                                     start=False, stop=True)
                    ut = stm.tile([128, 512], bf, tag="ut")
                    nc.scalar.activation(ut[:], pl[:], AF.Exp)
                    nc.tensor.matmul(den_acc[:], ones_cb[:], ut[:],
                                     start=(jc == 0), stop=(jc == NT - 1))
                    ut_tiles[jc] = ut
                    if jc >= 2:
                        emit_attn_acc(jc - 2)
            emit_kv(NT - 2)
            emit_kv(NT - 1)
            emit_attn_acc(NT - 2)
            emit_attn_acc(NT - 1)

            # ---------- tail: reordered for cross-engine overlap -------------
            aggloT = pers.tile([128, 2, SH], bf)
            tap_tiles["aggloT"] = aggloT
            g1T = pers.tile([128, 2, SH], bf)
            tgate = pers.tile([128, 2, SH], bf)
            tap_tiles["tgate"] = tgate
            h_localT = pers.tile([128, 2, SH], f32)
            tap_tiles["h_localT"] = h_localT
            qgzT = pers.tile([128, 2, SH], bf)
            tap_tiles["qgzT"] = qgzT
            yT = pers.tile([128, 2, SH], bf)
            tap_tiles["yT"] = yT
            h_globalT = pers.tile([128, 2, SH], f32)
            tap_tiles["h_globalT"] = h_globalT
            xoT = pers.tile([128, 2, SH], f32)
            tap_tiles["xoT"] = xoT
            xo_bf = pers.tile([128, 2, SH], bf)
            xnT = pers.tile([128, 2, SH], bf)
            tap_tiles["xnT"] = xnT
            ff1T = pers.tile([128, 8, SH], bf)
            outT = pers.tile([128, 2, SH], f32)
            tap_tiles["outT"] = outT

            # kv block-diagonal + ksum extraction (ACT/DVE, first thing)
            kvb = pers.tile([128, 2, D], bf)
            tap_tiles["kvb"] = kvb
            nc.vector.memset(kvb[:], 0.0)
            for h in range(H):
                g, po = h // 4, (h * DH) % 128
                nc.scalar.copy(kvb[po:po + DH, g, h * DH:(h + 1) * DH],
                               kv_ps[g][po:po + DH, h * DH:(h + 1) * DH])
            ksum_col = pers.tile([128, 2], f32)
            for g in range(2):
                nc.vector.tensor_copy(ksum_col[:, g:g + 1], kv_ps[g][:, D:D + 1])
            ksum_row = pers.tile([1, D], bf)
            for g in range(2):
                pt = psmm.tile([128, 128], f32, tag="mm")
                nc.tensor.transpose(pt[0:1, 0:128], ksum_col[:, g:g + 1], ident[:])
                nc.vector.tensor_copy(ksum_row[0:1, g * 128:(g + 1) * 128],
                                      pt[0:1, 0:128])
            kb_ps = psmm.tile([128, D], f32, tag="mm")
            nc.tensor.matmul(kb_ps[:], ones_rb[:], ksum_row[:], start=True, stop=True)
            ksumb = pers.tile([128, D], bf)
            tap_tiles["ksumb"] = ksumb
            nc.vector.tensor_copy(ksumb[:], kb_ps[:])

            # gate first-level matmuls on h (independent of agglo) keep PE busy
            pgs = [psacc.tile([128, 512], f32, tag="acc", name=f"pg{g}")
                   for g in range(2)]
            for g in range(2):
                for c in range(2):
                    nc.tensor.matmul(pgs[g][:], wg1[:, c, g * 128:(g + 1) * 128],
                                     hT[:, c, 0:SH], start=(c == 0), stop=False)

            # denominator reciprocal + agglo
            den_sb2 = stm.tile([1, 512], f32, tag="den_sb2")
            nc.vector.tensor_copy(den_sb2[:], den_acc[:])
            den_rb = pers.tile([1, 512], bf)
            tap_tiles["den_rb"] = den_rb
            with nc.allow_low_precision("bf16 recip feeds bf16 broadcast"):
                nc.vector.reciprocal(den_rb[:], den_sb2[:])
            rbp = psmm.tile([128, 512], f32, tag="mm", name="rbp")
            nc.tensor.matmul(rbp[:], ones_rb[:], den_rb[:], start=True, stop=True)
            rbh = stmf.tile([128, 512], bf, tag="tmpf", name="rbh")
            nc.scalar.copy(rbh[:], rbp[:])
            hl_base = pers.tile([128, 2, SH], f32)
            for g in range(2):
                nc.vector.tensor_mul(aggloT[:, g, :], agg_ps[g][:], rbh[:])
                nc.vector.scalar_tensor_tensor(hl_base[:, g, :], aggloT[:, g, :],
                                               0.5, hT[:, g, 0:SH],
                                               op0=ALU.mult, op1=ALU.add)

            # f1 chain matmuls + first gelu (triggers the one gelu table load)
            for g in range(2):
                pf = psmm.tile([128, 512], f32, tag="mm")
                for c in range(2):
                    nc.tensor.matmul(pf[:], wf1[:, c, g * 128:(g + 1) * 128],
                                     hT[:, c, 0:SH], start=(c == 0), stop=(c == 1))
                nc.scalar.activation(f1T[:, g, :], pf[:], AF.Gelu)

            # gate second half (agglo) + g1 gelu
            for g in range(2):
                for c in range(2):
                    nc.tensor.matmul(pgs[g][:], wg1[:, 2 + c, g * 128:(g + 1) * 128],
                                     aggloT[:, c, :], start=False, stop=(c == 1))
                nc.scalar.activation(g1T[:, g, :], pgs[g][:], AF.Gelu)

            # linear attention z + qgz (DVE) overlapping the gate/f1 chains
            zden_a = stm.tile([128, ST, H], f32, tag="zden_a")
            for it in range(ST):
                prod = stmq.tile([128, D], f32, tag="tmpq")
                nc.vector.tensor_mul(prod[:], qg_all[:, it, :], ksumb[:])
                nc.vector.tensor_reduce(zden_a[:, it, :],
                                        prod[:].rearrange("p (h d) -> p h d", d=DH),
                                        axis=AX.X, op=ALU.add)
            nc.vector.tensor_scalar_add(zden_a[:], zden_a[:], 1e-6)
            zr_a = stm.tile([128, ST, H], f32, tag="zr_a")
            nc.vector.reciprocal(
                zr_a[:].rearrange("p a b -> p (a b)"),
                zden_a[:].rearrange("p a b -> p (a b)"))
            qgz_t = []
            for it in range(ST):
                qgz = pers.tile([128, D], f32, name=f"qgz{it}")
                nc.vector.tensor_tensor(
                    out=qgz[:].rearrange("p (h d) -> p h d", d=DH),
                    in0=qg_all[:, it, :].rearrange("p (h d) -> p h d", d=DH),
                    in1=zr_a[:, it, :].to_broadcast([128, H, DH]), op=ALU.mult)
                qgz_t.append(qgz)
            # wf fuse-gate weights
            wf_ps = pssml.tile([2, 512], f32, tag="accs", name="wfps")
            wf_sb = stm.tile([2, 512], bf, tag="wf_sb")
            for c in range(2):
                nc.tensor.matmul(wf_ps[:], wf2[:, c, :], f1T[:, c, :],
                                 start=(c == 0), stop=(c == 1))
            nc.scalar.copy(wf_sb[:], wf_ps[:])
            d01_ps = psmm.tile([1, 512], f32, tag="mm", name="d01")
            nc.tensor.matmul(d01_ps[:], pm[:], wf_sb[:], start=True, stop=True)
            th_wf = stm.tile([1, 512], bf, tag="th_wf")
            nc.scalar.activation(th_wf[:], d01_ps[:], AF.Tanh, scale=0.5)
            wf0 = pers.tile([1, 512], bf)
            tap_tiles["wf0"] = wf0
            wf1s = pers.tile([1, 512], bf)
            nc.vector.tensor_scalar(wf0[:], th_wf[:], 0.5, 0.5,
                                    op0=ALU.mult, op1=ALU.add)
            nc.vector.tensor_scalar(wf1s[:], th_wf[:], -0.5, 0.5,
                                    op0=ALU.mult, op1=ALU.add)

            # gate second level + tanh; y and h_global in parallel
            for g in range(2):
                pg2 = psmm.tile([128, 512], f32, tag="mm")
                for c in range(2):
                    nc.tensor.matmul(pg2[:], wg2[:, c, g * 128:(g + 1) * 128],
                                     g1T[:, c, :], start=(c == 0), stop=(c == 1))
                nc.scalar.activation(tgate[:, g, :], pg2[:], AF.Tanh, scale=0.5)
            for it in range(ST):
                ts_ = slice(it * 128, (it + 1) * 128)
                for c in range(2):
                    ptq = psmm.tile([128, 128], f32, tag="mm")
                    nc.tensor.transpose(ptq[:], qgz_t[it][:, c * 128:(c + 1) * 128],
                                        ident[:])
                    nc.vector.tensor_copy(qgzT[:, c, ts_], ptq[:])
            for g in range(2):
                py = psmm.tile([128, 512], f32, tag="mm")
                for c in range(2):
                    nc.tensor.matmul(py[:], kvb[:, c, g * 128:(g + 1) * 128],
                                     qgzT[:, c, :], start=(c == 0), stop=(c == 1))
                nc.vector.tensor_copy(yT[:, g, :], py[:])
            for g in range(2):
                pgo = psmm.tile([128, 512], f32, tag="mm")
                for c in range(2):
                    nc.tensor.matmul(pgo[:], wgo[:, c, g * 128:(g + 1) * 128],
                                     yT[:, c, :], start=(c == 0), stop=(c == 1))
                nc.vector.tensor_add(h_globalT[:, g, :], hT[:, g, 0:SH], pgo[:])
            # h_local = (h + 0.5*agglo) + 0.5*(agglo*tanh)
            for g in range(2):
                w = stmf.tile([128, 512], bf, tag="tmpf")
                nc.vector.tensor_mul(w[:], tgate[:, g, :], aggloT[:, g, :])
                nc.vector.scalar_tensor_tensor(h_localT[:, g, :], w[:], 0.5,
                                               hl_base[:, g, :],
                                               op0=ALU.mult, op1=ALU.add)

            # xo = wf0*h_local + wf1*h_global
            b0p = psmm.tile([128, 512], f32, tag="mm", name="b0p")
            nc.tensor.matmul(b0p[:], ones_rb[:], wf0[:], start=True, stop=True)
            b1p = psmm.tile([128, 512], f32, tag="mm", name="b1p")
            nc.tensor.matmul(b1p[:], ones_rb[:], wf1s[:], start=True, stop=True)
            for g in range(2):
                ta = stmf.tile([128, 512], f32, tag="tmpf")
                nc.vector.tensor_mul(ta[:], h_localT[:, g, :], b0p[:])
                tb = stmf.tile([128, 512], f32, tag="tmpf")
                nc.vector.tensor_mul(tb[:], h_globalT[:, g, :], b1p[:])
                nc.vector.tensor_add(xoT[:, g, :], ta[:], tb[:])
                nc.vector.tensor_copy(xo_bf[:, g, :], xoT[:, g, :])

            # ---------- LN2: stats via matmul, rsqrt via column Heron ---------
            sum_ps = pssml.tile([1, 512], f32, tag="accs", name="sumps")
            for c in range(2):
                nc.tensor.matmul(sum_ps[:], ones_cb[:], xo_bf[:, c, :],
                                 start=(c == 0), stop=(c == 1))
            ssq_ps = psmm.tile([1, 512], f32, tag="mm", name="ssqps")
            for c in range(2):
                xsq = stmf.tile([128, 512], bf, tag="xsq")
                nc.scalar.activation(xsq[:], xoT[:, c, :], AF.Square)
                nc.tensor.matmul(ssq_ps[:], ones_cb[:], xsq[:],
                                 start=(c == 0), stop=(c == 1))
            sum_row = stm.tile([1, 512], f32, tag="sum_row")
            nc.vector.tensor_copy(sum_row[:], sum_ps[:])
            ssq_row = stm.tile([1, 512], f32, tag="ssq_row")
            nc.vector.tensor_copy(ssq_row[:], ssq_ps[:])
            sq_col = stm.tile([128, 4, 2], f32, tag="sq_col")
            for kk in range(4):
                pts = psmm.tile([128, 128], f32, tag="mm")
                nc.tensor.transpose(pts[0:128, 0:1],
                                    sum_row[0:1, kk * 128:(kk + 1) * 128],
                                    ident[0:1, 0:1])
                nc.vector.tensor_copy(sq_col[:, kk, 0:1], pts[0:128, 0:1])
                ptq2 = psmm.tile([128, 128], f32, tag="mm")
                nc.tensor.transpose(ptq2[0:128, 0:1],
                                    ssq_row[0:1, kk * 128:(kk + 1) * 128],
                                    ident[0:1, 0:1])
                nc.vector.tensor_copy(sq_col[:, kk, 1:2], ptq2[0:128, 0:1])
            mean_c = stm.tile([128, 4], f32, tag="mean_c")
            nc.vector.tensor_scalar_mul(mean_c[:], sq_col[:, :, 0], 1.0 / D)
            msq_c = stm.tile([128, 4], f32, tag="msq_c")
            nc.vector.tensor_scalar_mul(msq_c[:], sq_col[:, :, 1], 1.0 / D)
            rstd_c = stm.tile([128, 4], f32, tag="rstd_c")
            nmr_c = stm.tile([128, 4], f32, tag="nmr_c")
            heron_rstd(rstd_c[:], nmr_c[:], mean_c, msq_c, iters=5)
            nmr_row = stm.tile([1, 512], bf, tag="nmr_row")
            rstd_row = stm.tile([1, 512], bf, tag="rstd_row")
            for kk in range(4):
                ptb0 = psmm.tile([128, 128], f32, tag="mm")
                nc.tensor.transpose(ptb0[0:1, 0:128], nmr_c[:, kk:kk + 1],
                                    ident[:])
                nc.vector.tensor_copy(nmr_row[:, kk * 128:(kk + 1) * 128],
                                      ptb0[0:1, 0:128])
                ptb1 = psmm.tile([128, 128], f32, tag="mm")
                nc.tensor.transpose(ptb1[0:1, 0:128], rstd_c[:, kk:kk + 1],
                                    ident[:])
                nc.vector.tensor_copy(rstd_row[:, kk * 128:(kk + 1) * 128],
                                      ptb1[0:1, 0:128])
            nmr_b = psmm.tile([128, 512], f32, tag="mm", name="nmrb")
            nc.tensor.matmul(nmr_b[:], ones_rb[:], nmr_row[:],
                             start=True, stop=True)
            rb2_b = psmm.tile([128, 512], f32, tag="mm", name="rb2b")
            nc.tensor.matmul(rb2_b[:], ones_rb[:], rstd_row[:],
                             start=True, stop=True)
            for g in range(2):
                t1 = stmf.tile([128, 512], f32, tag="tmpf")
                nc.vector.tensor_mul(t1[:], xoT[:, g, :], rb2_b[:])
                # g2 is folded into Wff1 on the host; write bf16 directly
                nc.vector.tensor_add(xnT[:, g, :], t1[:], nmr_b[:])

            # FFN + residual + output transposes/stores
            pf2s = [psacc.tile([128, 512], f32, tag="acc", name=f"pf2_{g}")
                    for g in range(2)]
            for g8 in range(8):
                pff = psmm.tile([128, 512], f32, tag="mm")
                for c in range(2):
                    nc.tensor.matmul(pff[:], wff1[:, c, g8 * 128:(g8 + 1) * 128],
                                     xnT[:, c, :], start=(c == 0), stop=(c == 1))
                nc.scalar.activation(ff1T[:, g8, :], pff[:], AF.Gelu)
                if g8 >= 1:
                    for g in range(2):
                        nc.tensor.matmul(pf2s[g][:],
                                         wff2[:, g8 - 1, g * 128:(g + 1) * 128],
                                         ff1T[:, g8 - 1, :],
                                         start=(g8 == 1), stop=False)
            for g in range(2):
                nc.tensor.matmul(pf2s[g][:], wff2[:, 7, g * 128:(g + 1) * 128],
                                 ff1T[:, 7, :], start=False, stop=True)
                nc.vector.tensor_add(outT[:, g, :], xoT[:, g, :], pf2s[g][:])
            for it in range(ST):
                ts_ = slice(it * 128, (it + 1) * 128)
                ot = stmq.tile([128, D], f32, tag="tmpq")
                for c in range(2):
                    pto = psmm.tile([128, 128], f32, tag="mm")
                    nc.tensor.transpose(pto[:], outT[:, c, ts_], ident[:])
                    nc.vector.tensor_copy(ot[:, c * 128:(c + 1) * 128], pto[:])
                nc.sync.dma_start(out_d[ts_, :], ot[:])

            for name in taps:
                t = tap_tiles[name]
                td = nc.dram_tensor(f"tap_{name}", list(t.shape),
                                    t.dtype, kind="ExternalOutput")
                nc.sync.dma_start(td[:], t[:])

    nc.compile()
    return nc


def _host_prep(inputs):
    """Host-side preprocessing shared by all cores + per-core arrays."""
    x = np.asarray(inputs["x"], np.float32)
    mask = np.asarray(inputs["mask"])
    nbr_idx = np.asarray(inputs["nbr_idx"]).astype(np.int64)
    nbr_mask = np.asarray(inputs["nbr_mask"])
    rel_pos = np.asarray(inputs["rel_pos"]).astype(np.int64)

    if not (np.all(mask == 1)):
        raise NotImplementedError("kernel assumes mask == ones (spec fill)")

    # edge-bias table over the 65 possible rel values
    Erel = np.asarray(inputs["Erel"], np.float32)
    We1 = np.asarray(inputs["We1"], np.float32)
    be1 = np.asarray(inputs["be1"], np.float32)
    We2 = np.asarray(inputs["We2"], np.float32)
    be2 = np.asarray(inputs["be2"], np.float32)
    tab = (_gelu_np(Erel @ We1 + be1) @ We2 + be2)[:, 0]  # [65]

    rel = np.clip(rel_pos, -CLIP, CLIP) + CLIP
    ev = np.exp(tab[rel]) * (nbr_mask != 0)  # [B, L, K]

    # dense E^T per batch: ET[b][j, t] = sum_k ev[b,t,k] * [idx==j]
    ET = np.zeros((B, L, L), np.float32)
    for b in range(B):
        t_idx = np.repeat(np.arange(L), K)
        np.add.at(ET[b], (nbr_idx[b].ravel(), t_idx), ev[b].ravel())
    # log-domain (folded into the score PSUM in-kernel): -1e30 where empty
    LT = np.where(ET > 0, np.log(np.maximum(ET, 1e-30)), -1e30).astype(np.float32)

    aff = np.zeros((128, 2, 4), np.float32)
    for name, i in (("g1", 0), ("b1", 1), ("g2", 2), ("b2", 3)):
        v = np.asarray(inputs[name], np.float32)
        aff[:, :, i] = v.reshape(2, 128).T

    shared = {
        "aff": aff,
        "wq": _w_tiles(np.asarray(inputs["Wq"], np.float32) / 16.0, 2),
        "wk": _w_tiles(np.ascontiguousarray(
            np.asarray(inputs["Wk"], np.float32).T), 2),
        "wv": _w_tiles(np.asarray(inputs["Wv"], np.float32)
                       @ np.asarray(inputs["Wlo"], np.float32), 2),
        "wg1": _w_tiles(np.asarray(inputs["Wg1"], np.float32), 4),
        "wg2": _w_tiles(np.asarray(inputs["Wg2"], np.float32), 2),
        "wqkv": _w_tiles(np.asarray(inputs["Wqkv"], np.float32), 2),
        "wgo": _w_tiles(np.asarray(inputs["Wgo"], np.float32), 2),
        "wf1": _w_tiles(np.asarray(inputs["Wf1"], np.float32), 2),
        "wf2": _w_tiles(np.asarray(inputs["Wf2"], np.float32), 2),
        "wff1": _w_tiles(np.asarray(inputs["g2"], np.float32)[:, None]
                         * np.asarray(inputs["Wff1"], np.float32), 2),
        "wff2": _w_tiles(np.asarray(inputs["Wff2"], np.float32), 8),
        "pm": np.array([[1.0], [-1.0]], BF16),
    }
    for k in ("blo", "bg1", "bg2", "bf1", "bf2", "bff1", "bff2", "b2"):
        if not np.allclose(np.asarray(inputs[k]), 0.0):
            raise NotImplementedError(f"kernel assumes bias {k} == 0 (spec fill)")

    per_core = []
    for c in range(NCORES):
        b, s = c // SPB, c % SPB
        s0 = s * SH
        xp = np.roll(x[b], -s0, axis=0)
        ltp = np.roll(LT[b][:, s0:s0 + SH], -s0, axis=0).astype(BF16)
        per_core.append({"x": np.ascontiguousarray(xp),
                         "lt": np.ascontiguousarray(ltp)})
    return shared, per_core


def kernel(**inputs) -> np.ndarray:
    import concourse.bass_utils as bu

    if "nc" not in _CACHE:
        _CACHE["nc"] = _build()
    nc = _CACHE["nc"]

    shared, per_core = _host_prep(inputs)
    in_maps = [{**shared, **pc} for pc in per_core]
    res = bu.run_bass_kernel_spmd(nc, in_maps, core_ids=list(range(NCORES)))
    out = np.zeros((B, L, D), np.float32)
    for c in range(NCORES):
        b, s = c // SPB, c % SPB
        out[b, s * SH:(s + 1) * SH] = res.results[c]["out"]
    return out
